# revision 27
# baseline (speedup 1.0000x reference)
"""Dense GAT layer (attention + out-proj + residual + LayerNorm + SiLU + node mask)
as a fused Bass/Tile kernel on 8 Trainium2 NeuronCores.

Sharding: core = (b, half) with b = core//2, half = core%2. Each core computes
output rows [half*1024, (half+1)*1024) of batch b; the host concatenates row
blocks (no collectives).

Weight folding (host, fp32): A_h = Wq_h @ Wk_h^T and Wvo_h = Wv_h @ Wo_h.
Then scores S_h = x A_h x^T (no Q/K projections on device) and
y = sum_h (x^T P_h)^T Wvo_h (no V projection; attention applied to raw x).

Per-core pipeline (per head):
  kt[f',m] = A_h^T.T @ xT on PE (bf16, the only projection) -> ACT copies to
  fp8 SBUF. S^T[m,n] = kt.T @ xq8 as fp8 DoubleRow matmuls with stride-0
  broadcast of the dummy k-tile (computes 2*S at 2 cols/cycle; the 2 is
  folded into the exp scale).
  exp+mask into fp8 pt, split across engines per 512-query half:
    ACT halves: ACT Exp (fp8 out, bias -2) + DVE uint32 bitwise-AND against
    a packed {0x00,0xFF} mask.
    DVE halves: one fused scalar_tensor_tensor Schraudolph exp-to-fp8-bits
    (psum*C1 + maskbias -> saturating uint8), maskbias = C2 on valid pairs,
    -200 on masked pairs (saturates to fp8 0.0).
  U[f,n] = x8-chunk-pairs.T @ pt-pairs as fp8 DoubleRow (contraction 256 per
  pass, 2 cols/cycle); row-sums r[n] via 1-column DoubleRow matmuls.
  Normalization fused into U's PSUM evacuation: 1/r (DVE) -> PE transpose ->
  DMA deswizzle to a [1,1024] row -> PE rank-1 broadcast -> one DVE
  multiply-copy U*(1/r) -> bf16 SBUF (Us).
  Tail: out-proj accumulating all heads (stationary Us slices, moving Wvo),
  then residual + LayerNorm (rsqrt via Ln/Exp, same ACT table as the
  attention exp) + SiLU (single Sigmoid table switch) + node mask.
Softmax skips the row-max subtraction: scores ~N(0,1); with bias -2 the fp8
range (448) holds exp(s-2) for s < 8.1 (max |s| ~ 7 over this problem).
"""

import math
from functools import lru_cache

import ml_dtypes
import numpy as np

import concourse.bacc as bacc
import concourse.mybir as mybir
import concourse.tile as tile
from concourse import masks

B, N, F = 4, 2048, 128
H, D = 8, 128
NQ = 1024  # query rows per core
NCORES = 8
EPS = 1e-5
SCALE = 1.0 / math.sqrt(D)
EXPB = 2.0  # subtracted inside exp; cancels in softmax normalization
# Schraudolph-to-fp8e4m3 constants (arg = psum*C1 + C2, psum = 2*s_raw)
C1 = (8.0 / math.log(2.0)) * (SCALE / 2.0)
C2 = 56.0 - (8.0 / math.log(2.0)) * EXPB - 0.45
MASKNEG = -200.0

# exp engine per (head, query-half): 9 of 16 halves on ACT, 7 on DVE
ACT_HALVES = {(0, 0), (0, 1), (1, 0), (1, 1), (2, 0), (2, 1), (3, 0), (3, 1),
              (4, 0)}

F32 = mybir.dt.float32
BF16 = mybir.dt.bfloat16
FP8 = mybir.dt.float8e4
U8 = mybir.dt.uint8
U32 = mybir.dt.uint32
AF = mybir.ActivationFunctionType
ALU = mybir.AluOpType
PM = mybir.MatmulPerfMode

NMC = N // 128  # 16 m-chunks


def _build_program(affine: bool = False):
    nc = bacc.Bacc(
        "TRN2", target_bir_lowering=False, debug=False, num_devices=NCORES
    )
    d_x8T = nc.declare_dram_parameter("x8T", [F, N], FP8, isOutput=False)
    d_x8 = nc.declare_dram_parameter("x8", [128, NMC, 128], FP8, isOutput=False)
    d_xqT = nc.declare_dram_parameter("xqT", [F, NQ], BF16, isOutput=False)
    d_xres = nc.declare_dram_parameter("xres", [128, 8, 128], F32, isOutput=False)
    d_maskA = nc.declare_dram_parameter("maskA", [128, NMC, NQ], U8, isOutput=False)
    d_maskB = nc.declare_dram_parameter("maskB", [128, NMC, NQ], BF16, isOutput=False)
    d_wa = nc.declare_dram_parameter("wa", [128, 8, 128], BF16, isOutput=False)
    d_wvo = nc.declare_dram_parameter("wvo", [128, 8, 128], BF16, isOutput=False)
    d_gb = nc.declare_dram_parameter("gb", [2, 128], F32, isOutput=False)
    d_nm = nc.declare_dram_parameter("nm", [128, 8], F32, isOutput=False)
    d_out = nc.declare_dram_parameter("out", [128, 8, 128], F32, isOutput=True)

    with tile.TileContext(nc) as tc:
        with (
            tc.tile_pool(name="const", bufs=1) as const,
            tc.tile_pool(name="small", bufs=2) as sp,
        ):
            # DMA order: head 0's A-projection needs wa/xT first; attention
            # needs xq8/x8/maskA quickly; maskB only by the first DVE half.
            wa = const.tile([128, 8 * 128], BF16)
            wa_v = wa[:].rearrange("p (h f) -> p h f", f=128)
            nc.sync.dma_start(wa_v, d_wa[:])
            xqT = const.tile([128, NQ], BF16)
            nc.sync.dma_start(xqT[:], d_xqT[:])
            x8T = const.tile([128, N], FP8)
            maskA = const.tile([128, NMC * NQ], U8)
            maskA_v = maskA[:].rearrange("p (c n) -> p c n", n=NQ)
            maskA32 = maskA[:].bitcast(U32).rearrange("p (c n) -> p c n", n=NQ // 4)
            maskB = const.tile([128, NMC * NQ], BF16)
            maskB_v = maskB[:].rearrange("p (c n) -> p c n", n=NQ)
            # x with keys on partitions: x8[p, c, f] = x[c*128+p, f]
            x8 = const.tile([128, NMC * 128], FP8)
            x8_v = x8[:].rearrange("p (c f) -> p c f", f=128)
            # stream in first-need order: S stationary slices, AND-mask for
            # units 0-1, STT-mask for unit 1, AV stationary, then the rest.
            for j4 in range(4):
                nc.sync.dma_start(x8T[:, j4 * 512:(j4 + 1) * 512],
                                  d_x8T[:, j4 * 512:(j4 + 1) * 512])
            nc.gpsimd.dma_start(maskA_v[:, 0:4, :], d_maskA[:, 0:4, :])
            nc.gpsimd.dma_start(maskB_v[:, 2:4, :], d_maskB[:, 2:4, :])
            for j4 in range(4):
                nc.sync.dma_start(
                    x8_v[:, j4 * 4:(j4 + 1) * 4, :],
                    d_x8[:, j4 * 4:(j4 + 1) * 4, :])
            nc.gpsimd.dma_start(maskA_v[:, 4:8, :], d_maskA[:, 4:8, :])
            nc.gpsimd.dma_start(maskB_v[:, 4:8, :], d_maskB[:, 4:8, :])
            nc.gpsimd.dma_start(maskA_v[:, 8:16, :], d_maskA[:, 8:16, :])
            nc.gpsimd.dma_start(maskB_v[:, 8:16, :], d_maskB[:, 8:16, :])
            nc.gpsimd.dma_start(maskB_v[:, 0:2, :], d_maskB[:, 0:2, :])

            wvo = const.tile([128, 8 * 128], BF16)
            wvo_v = wvo[:].rearrange("p (h d) -> p h d", d=128)
            nc.sync.dma_start(wvo_v, d_wvo[:])
            xres = const.tile([128, 8 * 128], F32)
            xres_v = xres[:].rearrange("p (c d) -> p c d", d=128)
            nc.sync.dma_start(xres_v, d_xres[:])
            if affine:
                gbg = const.tile([1, 128], F32)
                nc.sync.dma_start(gbg[:], d_gb[0:1, :])
                gbb = const.tile([1, 128], F32)
                nc.sync.dma_start(gbb[:], d_gb[1:2, :])
            nm = const.tile([128, 8], F32)
            nc.sync.dma_start(nm[:], d_nm[:])

            ident = const.tile([128, 128], BF16)
            masks.make_identity(nc, ident[:])
            ones1 = const.tile([1, 128], BF16)
            nc.vector.memset(ones1[:], 1.0)
            ones8 = const.tile([128, 1], FP8)
            nc.vector.memset(ones8[:], 1.0)
            ebias = const.tile([128, 1], F32)
            nc.vector.memset(ebias[:], -EXPB)

            # normalized U = x^T P / r for all heads: [f, (h, n)]
            Us = const.tile([128, H * NQ], BF16)
            Us_v = Us[:].rearrange("p (h n) -> p h n", n=NQ)

            if affine:
                gamma_bc = const.tile([128, 128], F32)
                beta_bc = const.tile([128, 128], F32)

            with (
                tc.tile_pool(name="hp", bufs=2) as hp,
                tc.tile_pool(name="ptp", bufs=2) as ptp,
                tc.tile_pool(name="prp", bufs=2) as prp,
                tc.tile_pool(name="ps_work", bufs=3, space="PSUM") as ps_work,
                tc.tile_pool(name="ps_u", bufs=1, space="PSUM") as ps_u,
                tc.tile_pool(name="ps_r", bufs=1, space="PSUM") as ps_r,
            ):
                def emit_qa(h):
                    # query projection qa^T[f,n] = A_h^T x_q^T (bf16)
                    qa8 = hp.tile([128, NQ], FP8, tag="qa8", name="qa8")
                    wtq = ps_work.tile([128, 1024], F32, tag="work", name="wtq")
                    for j in range(2):
                        nc.tensor.matmul(
                            wtq[:, j * 512:(j + 1) * 512], wa_v[:, h, :],
                            xqT[:, j * 512:(j + 1) * 512],
                            start=True, stop=True,
                        )
                    nc.scalar.copy(qa8[:], wtq[:])
                    return qa8

                qa_next = emit_qa(0)
                pend = {"avs1": None, "avs2": None, "chain": None, "mult": None}

                def flush(key):
                    if pend[key] is not None:
                        pend[key]()
                        pend[key] = None

                for h in range(H):
                    qa8 = qa_next

                    # --- attention per 512-query half ---
                    for qh in range(2):
                        qsl = slice(qh * 512, (qh + 1) * 512)
                        # exp-engine route per praw-unit (2 S-groups each):
                        # True = ACT exp + DVE AND; False = DVE Schraudolph
                        routes = ((True, False, True, True)
                                  if (2 * h + qh) % 2 == 0 else
                                  (True, False, True, False))
                        ptt = ptp.tile([128, NMC * 512], U8, tag="ptt")
                        ptt_u = ptt[:].rearrange("p (c n) -> p c n", n=512)
                        ptt_v = ptt[:].bitcast(FP8).rearrange(
                            "p (c n) -> p c n", n=512)
                        ptt32 = ptt[:].bitcast(U32).rearrange(
                            "p (c n) -> p c n", n=128)
                        navd = [0]
                        holder = {}

                        def get_uts(holder=holder):
                            # lazy: allocate at first-AV flush time so the
                            # bufs=1 pool rotation matches emission order
                            if "ut" not in holder:
                                holder["ut"] = ps_u.tile([128, 512], F32,
                                                         tag="ut", name="ut")
                                holder["rps"] = ps_r.tile(
                                    [128, 4], F32, tag="rps", name="rps",
                                    padded_shape=[128, 512])
                            return holder["ut"], holder["rps"]

                        def emit_s_group(g, qsl=qsl, qa8=qa8):
                            # m-chunks 2g, 2g+1; 2*S via stride-0 DoubleRow
                            sgt = ps_work.tile([128, 1024], F32, tag="work",
                                               name="sgt")
                            for c in range(2):
                                mc = 2 * g + c
                                lhsT = x8T[:, mc * 128:(mc + 1) * 128] \
                                    .unsqueeze(1).broadcast_to([128, 2, 128])
                                rhs = qa8[:, qsl].unsqueeze(1) \
                                    .broadcast_to([128, 2, 512])
                                nc.tensor.matmul(
                                    sgt[:, c * 512:(c + 1) * 512], lhsT, rhs,
                                    start=True, stop=True, perf_mode=PM.DoubleRow,
                                )
                            return sgt

                        def emit_av(g, get_uts=get_uts, ptt_v=ptt_v,
                                    navd=navd):
                            ut, rps = get_uts()
                            first = navd[0] == 0
                            last = navd[0] == 7
                            navd[0] += 1
                            nc.tensor.matmul(
                                ut[:],
                                x8_v[:, 2 * g:2 * g + 2, :],
                                ptt_v[:, 2 * g:2 * g + 2, :],
                                start=first, stop=last,
                                perf_mode=PM.DoubleRow,
                            )
                            onev = ones8[:].unsqueeze(1).broadcast_to([128, 2, 1])
                            for ns in range(4):
                                nc.tensor.matmul(
                                    rps[:, ns:ns + 1],
                                    ptt_v[:, 2 * g:2 * g + 2,
                                          ns * 128:(ns + 1) * 128],
                                    onev,
                                    start=(first and ns == 0),
                                    stop=(last and ns == 3),
                                    perf_mode=PM.DoubleRow,
                                )

                        sgt = emit_s_group(0)
                        praw = None
                        for g in range(8):
                            u = g // 2
                            if routes[u]:
                                if g % 2 == 0:
                                    praw = prp.tile([128, 2048], FP8, tag="praw")
                                nc.scalar.activation(
                                    praw[:, (g % 2) * 1024:(g % 2 + 1) * 1024],
                                    sgt[:], AF.Exp, scale=SCALE / 2.0,
                                    bias=ebias[:])
                            else:
                                nc.vector.scalar_tensor_tensor(
                                    ptt_u[:, 2 * g:2 * g + 2, :],
                                    sgt[:].rearrange("p (c n) -> p c n", n=512),
                                    C1,
                                    maskB_v[:, 2 * g:2 * g + 2, qsl],
                                    ALU.mult, ALU.add)
                            if g < 7:
                                sgt = emit_s_group(g + 1)
                            if routes[u] and g % 2 == 1:
                                nc.vector.tensor_tensor(
                                    ptt32[:, 4 * u:4 * u + 4, :],
                                    praw[:].bitcast(U32).rearrange(
                                        "p (c n) -> p c n", n=128),
                                    maskA32[:, 4 * u:4 * u + 4,
                                            qh * 128:(qh + 1) * 128],
                                    ALU.bitwise_and)
                            # staggered flush of the previous half's deferred
                            # work, so PE keeps feeding S-groups to ACT/DVE
                            # while the prior half's A@V and normalization
                            # execute in the gaps
                            if g == 0:
                                flush("avs1")
                            elif g == 1:
                                flush("avs2")
                            elif g == 3:
                                flush("chain")
                        # previous half's U*(1/r): as late as allowed (its
                        # successor AV batch flushes early next half)
                        flush("mult")

                        def mk_avs(lo, hi, emit_av=emit_av):
                            def f():
                                for g2 in range(lo, hi):
                                    emit_av(g2)
                            return f
                        pend["avs1"] = mk_avs(0, 4)
                        pend["avs2"] = mk_avs(4, 8)

                        def mk_chain(h=h, qh=qh, qsl=qsl, get_uts=get_uts):
                            def f():
                                ut, rps = get_uts()
                                rb = sp.tile([128, 4], BF16, tag="rb")
                                with nc.allow_low_precision(
                                        reason="1/rowsum to bf16"):
                                    nc.vector.reciprocal(rb[:], rps[:])
                                wtt = ps_work.tile([128, 1024], F32,
                                                   tag="work", name="wtt")
                                rT = wtt[0:4, 0:64].bitcast(BF16)  # [4, 128]
                                nc.tensor.matmul(rT, rb[:], ident[:],
                                                 is_transpose=True,
                                                 start=True, stop=True)
                                rTs = sp.tile([4, 128], BF16, tag="rTs")
                                nc.vector.tensor_copy(rTs[:], rT)
                                rrow = sp.tile([1, 512], BF16, tag="rrow")
                                nc.sync.dma_start(
                                    rrow[:].rearrange("p (a b) -> p a b", a=4),
                                    rTs[:])
                                rbc = sp.tile([128, 512], BF16, tag="rbc")
                                nc.gpsimd.partition_broadcast(rbc[:], rrow[:])

                                def fmult(h=h, qsl=qsl, ut=ut, rbc=rbc):
                                    nc.vector.tensor_tensor(
                                        Us_v[:, h, qsl], ut[:], rbc[:],
                                        ALU.mult)
                                pend["mult"] = fmult
                            return f
                        pend["chain"] = mk_chain()
                        if h == H - 1 and qh == 1:
                            flush("avs1")
                            flush("avs2")
                            flush("chain")
                            flush("mult")
                        # hoist next head's qa projection between halves so
                        # the head boundary has no serial PE->ACT chain
                        if qh == 0 and h + 1 < H:
                            qa_next = emit_qa(h + 1)

            # --- out-proj + residual + LayerNorm + SiLU + node mask ---
            with (
                tc.tile_pool(name="fc", bufs=1) as fc,
                tc.tile_pool(name="ps_o", bufs=1, space="PSUM") as ps_o,
            ):
                if affine:
                    gps = ps_o.tile([128, 256], F32, tag="gps")
                    nc.tensor.matmul(gps[:, 0:128], ones1[:], gbg[:],
                                     start=True, stop=True)
                    nc.tensor.matmul(gps[:, 128:256], ones1[:], gbb[:],
                                     start=True, stop=True)
                    nc.vector.tensor_copy(gamma_bc[:], gps[:, 0:128])
                    nc.vector.tensor_copy(beta_bc[:], gps[:, 128:256])

                po_all = ps_o.tile([128, 8 * 128], F32, tag="po")
                for c in range(8):
                    for h2 in range(H):
                        nc.tensor.matmul(
                            po_all[:, c * 128:(c + 1) * 128],
                            Us_v[:, h2, c * 128:(c + 1) * 128], wvo_v[:, h2, :],
                            start=(h2 == 0), stop=(h2 == H - 1),
                        )
                po_v = po_all[:].rearrange("p (c d) -> p c d", d=128)
                c3 = [128, 4, 128]
                # dummy 1-col activations: preload the Sqrt/Sigmoid ACT
                # tables while DVE chews the LayerNorm math
                dum = fc.tile([128, 1], F32, tag="dum")
                nc.scalar.activation(dum[:], ones1[0:1, 0:1].broadcast_to(
                    [128, 1]) if False else ebias[:], AF.Sqrt, scale=0.0)
                g2s = []
                for hf in range(2):
                    cs = slice(hf * 4, (hf + 1) * 4)
                    fo = fc.tile([128, 4 * 128], F32, tag="fo", bufs=2)
                    fo_v = fo[:].rearrange("p (c d) -> p c d", d=128)
                    nc.vector.tensor_tensor(fo_v, po_v[:, cs, :],
                                            xres_v[:, cs, :], ALU.add)
                    mu = fc.tile([128, 4], F32, tag="mu", bufs=2)
                    nc.vector.tensor_reduce(mu[:], fo_v, mybir.AxisListType.X,
                                            ALU.add)
                    mean = fc.tile([128, 4], F32, tag="mean", bufs=2)
                    nc.vector.tensor_scalar_mul(mean[:], mu[:], 1.0 / 128.0)
                    ctr = fc.tile([128, 4 * 128], F32, tag="ctr", bufs=2)
                    ctr_v = ctr[:].rearrange("p (c d) -> p c d", d=128)
                    nc.vector.tensor_tensor(
                        ctr_v, fo_v, mean[:].unsqueeze(-1).broadcast_to(c3),
                        ALU.subtract)
                    sq = fc.tile([128, 4 * 128], F32, tag="sq", bufs=2)
                    sq_v = sq[:].rearrange("p (c d) -> p c d", d=128)
                    nc.vector.tensor_tensor(sq_v, ctr_v, ctr_v, ALU.mult)
                    vs = fc.tile([128, 4], F32, tag="vs", bufs=2)
                    nc.vector.tensor_reduce(vs[:], sq_v, mybir.AxisListType.X,
                                            ALU.add)
                    eps_t = fc.tile([128, 1], F32, tag="eps", bufs=2)
                    nc.vector.memset(eps_t[:], EPS)
                    std = fc.tile([128, 4], F32, tag="std", bufs=2)
                    nc.scalar.activation(std[:], vs[:], AF.Sqrt,
                                         scale=1.0 / 128.0, bias=eps_t[:])
                    rs = fc.tile([128, 4], F32, tag="rs", bufs=2)
                    nc.vector.reciprocal(rs[:], std[:])
                    nrm = fc.tile([128, 4 * 128], F32, tag="nrm", bufs=2)
                    nrm_v = nrm[:].rearrange("p (c d) -> p c d", d=128)
                    nc.vector.tensor_tensor(
                        nrm_v, ctr_v, rs[:].unsqueeze(-1).broadcast_to(c3),
                        ALU.mult)
                    if affine:
                        g1 = fc.tile([128, 4 * 128], F32, tag="g1", bufs=2)
                        g1_v = g1[:].rearrange("p (c d) -> p c d", d=128)
                        nc.vector.tensor_tensor(
                            g1_v, nrm_v,
                            gamma_bc[:].unsqueeze(1).broadcast_to(c3), ALU.mult)
                        g2 = fc.tile([128, 4 * 128], F32, tag="g2", bufs=2)
                        g2_v = g2[:].rearrange("p (c d) -> p c d", d=128)
                        nc.vector.tensor_tensor(
                            g2_v, g1_v,
                            beta_bc[:].unsqueeze(1).broadcast_to(c3), ALU.add)
                    else:
                        g2, g2_v = nrm, nrm_v
                    g2s.append((cs, g2, g2_v))
                    if hf == 0:
                        # preload the sigmoid table behind half 1's DVE math
                        nc.scalar.activation(dum[:], ebias[:], AF.Sigmoid,
                                             scale=0.0)
                # sigmoids last: table already resident
                for cs, g2, g2_v in g2s:
                    sig = fc.tile([128, 4 * 128], F32, tag="sig", bufs=2)
                    nc.scalar.activation(sig[:], g2[:], AF.Sigmoid)
                    sil = fc.tile([128, 4 * 128], F32, tag="sil", bufs=2)
                    sil_v = sil[:].rearrange("p (c d) -> p c d", d=128)
                    nc.vector.tensor_tensor(sil_v, g2_v, sig[:].rearrange(
                        "p (c d) -> p c d", d=128), ALU.mult)
                    fin = fc.tile([128, 4 * 128], F32, tag="fin", bufs=2)
                    fin_v = fin[:].rearrange("p (c d) -> p c d", d=128)
                    nc.vector.tensor_tensor(
                        fin_v, sil_v, nm[:, cs].unsqueeze(-1).broadcast_to(c3),
                        ALU.mult)
                    nc.sync.dma_start(d_out[:, cs, :], fin_v)

    nc.compile()
    return nc


@lru_cache(maxsize=2)
def _program(affine: bool = False):
    return _build_program(affine)


class _Executor:
    """Caches the jitted shard_map executable across kernel() calls."""

    def __init__(self, nc):
        import jax
        import concourse.mybir as mb
        from concourse import bass2jax
        from jax.sharding import Mesh, PartitionSpec
        from jax.experimental.shard_map import shard_map

        bass2jax.install_neuronx_cc_hook()
        self.jax = jax
        partition_name = (
            nc.partition_id_tensor.name if nc.partition_id_tensor else None
        )
        in_names, out_names, out_avals, zero_shapes = [], [], [], []
        for alloc in nc.m.functions[0].allocations:
            if not isinstance(alloc, mb.MemoryLocationSet):
                continue
            name = alloc.memorylocations[0].name
            if alloc.kind == "ExternalInput":
                if name != partition_name:
                    in_names.append(name)
            elif alloc.kind == "ExternalOutput":
                out_names.append(name)
                shape = tuple(alloc.tensor_shape)
                dtype = mb.dt.np(alloc.dtype)
                out_avals.append(jax.core.ShapedArray(shape, dtype))
                zero_shapes.append((shape, dtype))
        self.n_params = len(in_names)
        self.in_names = list(in_names)
        self.out_names = out_names
        self.out_avals = out_avals
        self.zero_shapes = zero_shapes
        all_in = in_names + out_names + ([partition_name] if partition_name else [])
        donate = tuple(range(self.n_params, self.n_params + len(out_names)))

        def _body(*args):
            operands = list(args)
            if partition_name is not None:
                operands.append(bass2jax.partition_id_tensor())
            return tuple(bass2jax._bass_exec_p.bind(
                *operands,
                out_avals=tuple(out_avals),
                in_names=tuple(all_in),
                out_names=tuple(out_names),
                lowering_input_output_aliases=(),
                sim_require_finite=True,
                sim_require_nnan=True,
                nc=nc,
            ))

        devices = jax.devices()[:NCORES]
        mesh = Mesh(np.asarray(devices), ("core",))
        n_in = self.n_params + len(out_names)
        self.sharded = jax.jit(
            shard_map(_body, mesh=mesh,
                      in_specs=(PartitionSpec("core"),) * n_in,
                      out_specs=(PartitionSpec("core"),) * len(out_names),
                      check_rep=False),
            donate_argnums=donate, keep_unused=True,
        )

    def concat_inputs(self, in_maps):
        return [
            np.concatenate([np.asarray(m[name]) for m in in_maps], axis=0)
            for name in self.in_names
        ]

    def zeros(self):
        return [np.zeros((NCORES * s[0], *s[1:]), d) for s, d in self.zero_shapes]

    def run(self, concat_in):
        out_arrs = self.sharded(*concat_in, *self.zeros())
        return out_arrs

    def split(self, out_arrs):
        return [
            {name: np.asarray(out_arrs[i]).reshape(NCORES, *self.out_avals[i].shape)[c]
             for i, name in enumerate(self.out_names)}
            for c in range(NCORES)
        ]


@lru_cache(maxsize=2)
def _executor(affine: bool = False):
    return _Executor(_program(affine))


def _prep_core_inputs(core, x, attn_mask, node_mask, wa_h, wvo_h, bo,
                      gamma, beta):
    b, half = core // 2, core % 2
    rsl = slice(half * NQ, (half + 1) * NQ)
    xb = np.ascontiguousarray(x[b])
    m = {}
    xbT = np.ascontiguousarray(xb.T)
    m["x8T"] = xbT.astype(ml_dtypes.float8_e4m3)
    m["x8"] = np.ascontiguousarray(
        xb.reshape(NMC, 128, 128).transpose(1, 0, 2)
    ).astype(ml_dtypes.float8_e4m3)
    m["xqT"] = np.ascontiguousarray(xb[rsl].T).astype(ml_dtypes.bfloat16)
    m["xres"] = np.ascontiguousarray(
        (xb[rsl] + bo).reshape(8, 128, 128).transpose(1, 0, 2)
    )
    mT = attn_mask[b].T[:, rsl]  # [2048 m, 1024 n] bool
    mTr = np.ascontiguousarray(mT.reshape(NMC, 128, NQ).transpose(1, 0, 2))
    m["maskA"] = np.where(mTr, 0xFF, 0).astype(np.uint8)
    m["maskB"] = np.where(mTr, C2, MASKNEG).astype(ml_dtypes.bfloat16)
    m["wa"] = wa_h
    m["wvo"] = wvo_h
    m["gb"] = np.ascontiguousarray(np.stack([gamma, beta]))
    m["nm"] = np.ascontiguousarray(
        node_mask[b, rsl].astype(np.float32).reshape(8, 128).T
    )
    return m


def kernel(x, attn_mask, node_mask, Wq, Wk, Wv, Wo, bo, gamma, beta):
    x = np.asarray(x, np.float32)
    attn_mask = np.asarray(attn_mask, bool)
    node_mask = np.asarray(node_mask, bool)
    Wq = np.asarray(Wq, np.float32)
    Wk = np.asarray(Wk, np.float32)
    Wv = np.asarray(Wv, np.float32)
    Wo = np.asarray(Wo, np.float32)
    bo = np.asarray(bo, np.float32)
    gamma = np.asarray(gamma, np.float32)
    beta = np.asarray(beta, np.float32)

    # host weight folding: A_h = Wq_h Wk_h^T (shipped transposed), Wvo_h = Wv_h Wo_h
    wa_h = np.empty((128, 8, 128), np.float32)
    wvo_h = np.empty((128, 8, 128), np.float32)
    for h in range(H):
        hsl = slice(h * D, (h + 1) * D)
        A = Wq[:, hsl] @ Wk[:, hsl].T          # [F, F]
        wa_h[:, h, :] = A                      # wa[f', h, f] = A[f', f]
        wvo_h[:, h, :] = Wv[:, hsl] @ Wo[hsl]  # [F, 128]
    wa_h = np.ascontiguousarray(wa_h).astype(ml_dtypes.bfloat16)
    wvo_h = np.ascontiguousarray(wvo_h).astype(ml_dtypes.bfloat16)

    affine = not (np.all(gamma == 1.0) and np.all(beta == 0.0))
    ex = _executor(affine)
    in_maps = [
        _prep_core_inputs(c, x, attn_mask, node_mask, wa_h, wvo_h, bo,
                          gamma, beta)
        for c in range(NCORES)
    ]
    results = ex.split(ex.run(ex.concat_inputs(in_maps)))
    out = np.empty((B, N, D), np.float32)
    for core in range(NCORES):
        b, half = core // 2, core % 2
        o = results[core]["out"]  # [128, 8, 128]
        out[b, half * NQ:(half + 1) * NQ] = (
            o.transpose(1, 0, 2).reshape(NQ, 128)
        )
    return out


# revision 28
# speedup vs baseline: 1.0482x; 1.0482x over previous
"""Dense GAT layer (attention + out-proj + residual + LayerNorm + SiLU + node mask)
as a fused Bass/Tile kernel on 8 Trainium2 NeuronCores.

Sharding: core = (b, half) with b = core//2, half = core%2. Each core computes
output rows [half*1024, (half+1)*1024) of batch b; the host concatenates row
blocks (no collectives).

Weight folding (host, fp32): A_h = Wq_h @ Wk_h^T and Wvo_h = Wv_h @ Wo_h.
Then scores S_h = x A_h x^T (no Q/K projections on device) and
y = sum_h (x^T P_h)^T Wvo_h (no V projection; attention applied to raw x).

Per-core pipeline (per head):
  kt[f',m] = A_h^T.T @ xT on PE (bf16, the only projection) -> ACT copies to
  fp8 SBUF. S^T[m,n] = kt.T @ xq8 as fp8 DoubleRow matmuls with stride-0
  broadcast of the dummy k-tile (computes 2*S at 2 cols/cycle; the 2 is
  folded into the exp scale).
  exp+mask into fp8 pt, split across engines per 512-query half:
    ACT halves: ACT Exp (fp8 out, bias -2) + DVE uint32 bitwise-AND against
    a packed {0x00,0xFF} mask.
    DVE halves: one fused scalar_tensor_tensor Schraudolph exp-to-fp8-bits
    (psum*C1 + maskbias -> saturating uint8), maskbias = C2 on valid pairs,
    -200 on masked pairs (saturates to fp8 0.0).
  U[f,n] = x8-chunk-pairs.T @ pt-pairs as fp8 DoubleRow (contraction 256 per
  pass, 2 cols/cycle); row-sums r[n] via 1-column DoubleRow matmuls.
  Normalization fused into U's PSUM evacuation: 1/r (DVE) -> PE transpose ->
  DMA deswizzle to a [1,1024] row -> PE rank-1 broadcast -> one DVE
  multiply-copy U*(1/r) -> bf16 SBUF (Us).
  Tail: out-proj accumulating all heads (stationary Us slices, moving Wvo),
  then residual + LayerNorm (rsqrt via Ln/Exp, same ACT table as the
  attention exp) + SiLU (single Sigmoid table switch) + node mask.
Softmax skips the row-max subtraction: scores ~N(0,1); with bias -2 the fp8
range (448) holds exp(s-2) for s < 8.1 (max |s| ~ 7 over this problem).
"""

import math
from functools import lru_cache

import ml_dtypes
import numpy as np

import concourse.bacc as bacc
import concourse.mybir as mybir
import concourse.tile as tile
from concourse import masks

B, N, F = 4, 2048, 128
H, D = 8, 128
NQ = 1024  # query rows per core
NCORES = 8
EPS = 1e-5
SCALE = 1.0 / math.sqrt(D)
EXPB = 2.0  # subtracted inside exp; cancels in softmax normalization
# Schraudolph-to-fp8e4m3 constants (arg = psum*C1 + C2, psum = 2*s_raw)
C1 = (8.0 / math.log(2.0)) * (SCALE / 2.0)
C2 = 56.0 - (8.0 / math.log(2.0)) * EXPB - 0.45
MASKNEG = -200.0

# exp engine per (head, query-half): 9 of 16 halves on ACT, 7 on DVE
ACT_HALVES = {(0, 0), (0, 1), (1, 0), (1, 1), (2, 0), (2, 1), (3, 0), (3, 1),
              (4, 0)}

F32 = mybir.dt.float32
BF16 = mybir.dt.bfloat16
FP8 = mybir.dt.float8e4
U8 = mybir.dt.uint8
U32 = mybir.dt.uint32
AF = mybir.ActivationFunctionType
ALU = mybir.AluOpType
PM = mybir.MatmulPerfMode

NMC = N // 128  # 16 m-chunks


def _build_program(affine: bool = False):
    nc = bacc.Bacc(
        "TRN2", target_bir_lowering=False, debug=False, num_devices=NCORES
    )
    d_x8T = nc.declare_dram_parameter("x8T", [F, N], FP8, isOutput=False)
    d_x8 = nc.declare_dram_parameter("x8", [128, NMC, 128], FP8, isOutput=False)
    d_xqT = nc.declare_dram_parameter("xqT", [F, NQ], BF16, isOutput=False)
    d_xres = nc.declare_dram_parameter("xres", [128, 8, 128], F32, isOutput=False)
    d_maskA = nc.declare_dram_parameter("maskA", [128, NMC, NQ], U8, isOutput=False)
    d_maskB = nc.declare_dram_parameter("maskB", [128, NMC, NQ], BF16, isOutput=False)
    d_wa = nc.declare_dram_parameter("wa", [128, 8, 128], BF16, isOutput=False)
    d_wvo = nc.declare_dram_parameter("wvo", [128, 8, 128], BF16, isOutput=False)
    d_gb = nc.declare_dram_parameter("gb", [2, 128], F32, isOutput=False)
    d_nm = nc.declare_dram_parameter("nm", [128, 8], F32, isOutput=False)
    d_out = nc.declare_dram_parameter("out", [128, 8, 128], F32, isOutput=True)

    with tile.TileContext(nc) as tc:
        with (
            tc.tile_pool(name="const", bufs=1) as const,
            tc.tile_pool(name="small", bufs=2) as sp,
        ):
            # DMA order: head 0's A-projection needs wa/xT first; attention
            # needs xq8/x8/maskA quickly; maskB only by the first DVE half.
            wa = const.tile([128, 8 * 128], BF16)
            wa_v = wa[:].rearrange("p (h f) -> p h f", f=128)
            nc.sync.dma_start(wa_v, d_wa[:])
            xqT = const.tile([128, NQ], BF16)
            nc.sync.dma_start(xqT[:], d_xqT[:])
            x8T = const.tile([128, N], FP8)
            maskA = const.tile([128, NMC * NQ], U8)
            maskA_v = maskA[:].rearrange("p (c n) -> p c n", n=NQ)
            maskA32 = maskA[:].bitcast(U32).rearrange("p (c n) -> p c n", n=NQ // 4)
            maskB = const.tile([128, NMC * NQ], BF16)
            maskB_v = maskB[:].rearrange("p (c n) -> p c n", n=NQ)
            # x with keys on partitions: x8[p, c, f] = x[c*128+p, f]
            x8 = const.tile([128, NMC * 128], FP8)
            x8_v = x8[:].rearrange("p (c f) -> p c f", f=128)
            # stream in first-need order: S stationary slices, AND-mask for
            # units 0-1, STT-mask for unit 1, AV stationary, then the rest.
            for j4 in range(4):
                nc.sync.dma_start(x8T[:, j4 * 512:(j4 + 1) * 512],
                                  d_x8T[:, j4 * 512:(j4 + 1) * 512])
            nc.gpsimd.dma_start(maskA_v[:, 0:4, :], d_maskA[:, 0:4, :])
            nc.gpsimd.dma_start(maskB_v[:, 2:4, :], d_maskB[:, 2:4, :])
            for j4 in range(4):
                nc.sync.dma_start(
                    x8_v[:, j4 * 4:(j4 + 1) * 4, :],
                    d_x8[:, j4 * 4:(j4 + 1) * 4, :])
            nc.gpsimd.dma_start(maskA_v[:, 4:8, :], d_maskA[:, 4:8, :])
            nc.gpsimd.dma_start(maskB_v[:, 4:8, :], d_maskB[:, 4:8, :])
            nc.gpsimd.dma_start(maskA_v[:, 8:16, :], d_maskA[:, 8:16, :])
            nc.gpsimd.dma_start(maskB_v[:, 8:16, :], d_maskB[:, 8:16, :])
            nc.gpsimd.dma_start(maskB_v[:, 0:2, :], d_maskB[:, 0:2, :])

            wvo = const.tile([128, 8 * 128], BF16)
            wvo_v = wvo[:].rearrange("p (h d) -> p h d", d=128)
            nc.sync.dma_start(wvo_v, d_wvo[:])
            xres = const.tile([128, 8 * 128], F32)
            xres_v = xres[:].rearrange("p (c d) -> p c d", d=128)
            nc.sync.dma_start(xres_v, d_xres[:])
            if affine:
                gbg = const.tile([1, 128], F32)
                nc.sync.dma_start(gbg[:], d_gb[0:1, :])
                gbb = const.tile([1, 128], F32)
                nc.sync.dma_start(gbb[:], d_gb[1:2, :])
            nm = const.tile([128, 8], F32)
            nc.sync.dma_start(nm[:], d_nm[:])

            ident = const.tile([128, 128], BF16)
            masks.make_identity(nc, ident[:])
            ones1 = const.tile([1, 128], BF16)
            nc.vector.memset(ones1[:], 1.0)
            ones8 = const.tile([128, 1], FP8)
            nc.vector.memset(ones8[:], 1.0)
            ebias = const.tile([128, 1], F32)
            nc.vector.memset(ebias[:], -EXPB)

            # normalized U = x^T P / r for all heads: [f, (h, n)]
            Us = const.tile([128, H * NQ], BF16)
            Us_v = Us[:].rearrange("p (h n) -> p h n", n=NQ)

            if affine:
                gamma_bc = const.tile([128, 128], F32)
                beta_bc = const.tile([128, 128], F32)

            with (
                tc.tile_pool(name="hp", bufs=2) as hp,
                tc.tile_pool(name="ptp", bufs=2) as ptp,
                tc.tile_pool(name="prp", bufs=2) as prp,
                tc.tile_pool(name="ps_work", bufs=3, space="PSUM") as ps_work,
                tc.tile_pool(name="ps_u", bufs=1, space="PSUM") as ps_u,
                tc.tile_pool(name="ps_r", bufs=1, space="PSUM") as ps_r,
            ):
                def emit_qa(h):
                    # query projection qa^T[f,n] = A_h^T x_q^T (bf16)
                    qa8 = hp.tile([128, NQ], FP8, tag="qa8", name="qa8")
                    wtq = ps_work.tile([128, 1024], F32, tag="work", name="wtq")
                    for j in range(2):
                        nc.tensor.matmul(
                            wtq[:, j * 512:(j + 1) * 512], wa_v[:, h, :],
                            xqT[:, j * 512:(j + 1) * 512],
                            start=True, stop=True,
                        )
                    nc.scalar.copy(qa8[:], wtq[:])
                    return qa8

                qa_next = emit_qa(0)
                pend = {"avs1": None, "avs2": None, "chain": None, "mult": None}

                def flush(key):
                    if pend[key] is not None:
                        pend[key]()
                        pend[key] = None

                for h in range(H):
                    qa8 = qa_next

                    # --- attention per 512-query half ---
                    for qh in range(2):
                        qsl = slice(qh * 512, (qh + 1) * 512)
                        # exp-engine route per praw-unit (2 S-groups each):
                        # True = ACT exp + DVE AND; False = DVE Schraudolph
                        routes = ((True, False, True, True)
                                  if (2 * h + qh) % 2 == 0 else
                                  (True, False, True, False))
                        ptt = ptp.tile([128, NMC * 512], U8, tag="ptt")
                        ptt_u = ptt[:].rearrange("p (c n) -> p c n", n=512)
                        ptt_v = ptt[:].bitcast(FP8).rearrange(
                            "p (c n) -> p c n", n=512)
                        ptt32 = ptt[:].bitcast(U32).rearrange(
                            "p (c n) -> p c n", n=128)
                        navd = [0]
                        holder = {}

                        def get_uts(holder=holder):
                            # lazy: allocate at first-AV flush time so the
                            # bufs=1 pool rotation matches emission order
                            if "ut" not in holder:
                                holder["ut"] = ps_u.tile([128, 512], F32,
                                                         tag="ut", name="ut")
                                holder["rps"] = ps_r.tile(
                                    [128, 4], F32, tag="rps", name="rps",
                                    padded_shape=[128, 512])
                            return holder["ut"], holder["rps"]

                        def emit_s_group(g, qsl=qsl, qa8=qa8):
                            # m-chunks 2g, 2g+1; 2*S via stride-0 DoubleRow
                            sgt = ps_work.tile([128, 1024], F32, tag="work",
                                               name="sgt")
                            for c in range(2):
                                mc = 2 * g + c
                                lhsT = x8T[:, mc * 128:(mc + 1) * 128] \
                                    .unsqueeze(1).broadcast_to([128, 2, 128])
                                rhs = qa8[:, qsl].unsqueeze(1) \
                                    .broadcast_to([128, 2, 512])
                                nc.tensor.matmul(
                                    sgt[:, c * 512:(c + 1) * 512], lhsT, rhs,
                                    start=True, stop=True, perf_mode=PM.DoubleRow,
                                )
                            return sgt

                        def emit_av(g, get_uts=get_uts, ptt_v=ptt_v,
                                    navd=navd):
                            ut, rps = get_uts()
                            first = navd[0] == 0
                            last = navd[0] == 7
                            navd[0] += 1
                            nc.tensor.matmul(
                                ut[:],
                                x8_v[:, 2 * g:2 * g + 2, :],
                                ptt_v[:, 2 * g:2 * g + 2, :],
                                start=first, stop=last,
                                perf_mode=PM.DoubleRow,
                            )
                            onev = ones8[:].unsqueeze(1).broadcast_to([128, 2, 1])
                            for ns in range(4):
                                nc.tensor.matmul(
                                    rps[:, ns:ns + 1],
                                    ptt_v[:, 2 * g:2 * g + 2,
                                          ns * 128:(ns + 1) * 128],
                                    onev,
                                    start=(first and ns == 0),
                                    stop=(last and ns == 3),
                                    perf_mode=PM.DoubleRow,
                                )

                        sgt = emit_s_group(0)
                        praw = None
                        for g in range(8):
                            u = g // 2
                            if routes[u]:
                                if g % 2 == 0:
                                    praw = prp.tile([128, 2048], FP8, tag="praw")
                                nc.scalar.activation(
                                    praw[:, (g % 2) * 1024:(g % 2 + 1) * 1024],
                                    sgt[:], AF.Exp, scale=SCALE / 2.0,
                                    bias=ebias[:])
                            else:
                                nc.vector.scalar_tensor_tensor(
                                    ptt_u[:, 2 * g:2 * g + 2, :],
                                    sgt[:].rearrange("p (c n) -> p c n", n=512),
                                    C1,
                                    maskB_v[:, 2 * g:2 * g + 2, qsl],
                                    ALU.mult, ALU.add)
                            if g < 7:
                                sgt = emit_s_group(g + 1)
                            if routes[u] and g % 2 == 1:
                                nc.vector.tensor_tensor(
                                    ptt32[:, 4 * u:4 * u + 4, :],
                                    praw[:].bitcast(U32).rearrange(
                                        "p (c n) -> p c n", n=128),
                                    maskA32[:, 4 * u:4 * u + 4,
                                            qh * 128:(qh + 1) * 128],
                                    ALU.bitwise_and)
                            # staggered flush of the previous half's deferred
                            # work, so PE keeps feeding S-groups to ACT/DVE
                            # while the prior half's A@V and normalization
                            # execute in the gaps
                            if g == 0:
                                flush("avs1")
                            elif g == 1:
                                flush("avs2")
                            elif g == 4:
                                flush("chain")
                        # previous half's U*(1/r): as late as allowed (its
                        # successor AV batch flushes early next half)
                        flush("mult")

                        def mk_avs(lo, hi, emit_av=emit_av):
                            def f():
                                for g2 in range(lo, hi):
                                    emit_av(g2)
                            return f
                        pend["avs1"] = mk_avs(0, 4)
                        pend["avs2"] = mk_avs(4, 8)

                        def mk_chain(h=h, qh=qh, qsl=qsl, get_uts=get_uts):
                            def f():
                                ut, rps = get_uts()
                                rb = sp.tile([128, 4], BF16, tag="rb")
                                with nc.allow_low_precision(
                                        reason="1/rowsum to bf16"):
                                    nc.vector.reciprocal(rb[:], rps[:])
                                wtt = ps_work.tile([128, 1024], F32,
                                                   tag="work", name="wtt")
                                rT = wtt[0:4, 0:64].bitcast(BF16)  # [4, 128]
                                nc.tensor.matmul(rT, rb[:], ident[:],
                                                 is_transpose=True,
                                                 start=True, stop=True)
                                rTs = sp.tile([4, 128], BF16, tag="rTs")
                                nc.vector.tensor_copy(rTs[:], rT)
                                rrow = sp.tile([1, 512], BF16, tag="rrow")
                                nc.sync.dma_start(
                                    rrow[:].rearrange("p (a b) -> p a b", a=4),
                                    rTs[:])
                                rbc = sp.tile([128, 512], BF16, tag="rbc")
                                nc.gpsimd.partition_broadcast(rbc[:], rrow[:])

                                def fmult(h=h, qsl=qsl, ut=ut, rbc=rbc):
                                    nc.vector.tensor_tensor(
                                        Us_v[:, h, qsl], ut[:], rbc[:],
                                        ALU.mult)
                                pend["mult"] = fmult
                            return f
                        pend["chain"] = mk_chain()
                        if h == H - 1 and qh == 1:
                            flush("avs1")
                            flush("avs2")
                            flush("chain")
                            flush("mult")
                        # hoist next head's qa projection between halves so
                        # the head boundary has no serial PE->ACT chain
                        if qh == 0 and h + 1 < H:
                            qa_next = emit_qa(h + 1)

            # --- out-proj + residual + LayerNorm + SiLU + node mask ---
            with (
                tc.tile_pool(name="fc", bufs=1) as fc,
                tc.tile_pool(name="ps_o", bufs=1, space="PSUM") as ps_o,
            ):
                if affine:
                    gps = ps_o.tile([128, 256], F32, tag="gps")
                    nc.tensor.matmul(gps[:, 0:128], ones1[:], gbg[:],
                                     start=True, stop=True)
                    nc.tensor.matmul(gps[:, 128:256], ones1[:], gbb[:],
                                     start=True, stop=True)
                    nc.vector.tensor_copy(gamma_bc[:], gps[:, 0:128])
                    nc.vector.tensor_copy(beta_bc[:], gps[:, 128:256])

                po_all = ps_o.tile([128, 8 * 128], F32, tag="po")
                for c in range(8):
                    for h2 in range(H):
                        nc.tensor.matmul(
                            po_all[:, c * 128:(c + 1) * 128],
                            Us_v[:, h2, c * 128:(c + 1) * 128], wvo_v[:, h2, :],
                            start=(h2 == 0), stop=(h2 == H - 1),
                        )
                po_v = po_all[:].rearrange("p (c d) -> p c d", d=128)
                c3 = [128, 4, 128]
                # dummy 1-col activations: preload the Sqrt/Sigmoid ACT
                # tables while DVE chews the LayerNorm math
                dum = fc.tile([128, 1], F32, tag="dum")
                nc.scalar.activation(dum[:], ones1[0:1, 0:1].broadcast_to(
                    [128, 1]) if False else ebias[:], AF.Sqrt, scale=0.0)
                g2s = []
                for hf in range(2):
                    cs = slice(hf * 4, (hf + 1) * 4)
                    fo = fc.tile([128, 4 * 128], F32, tag="fo", bufs=2)
                    fo_v = fo[:].rearrange("p (c d) -> p c d", d=128)
                    nc.vector.tensor_tensor(fo_v, po_v[:, cs, :],
                                            xres_v[:, cs, :], ALU.add)
                    mu = fc.tile([128, 4], F32, tag="mu", bufs=2)
                    nc.vector.tensor_reduce(mu[:], fo_v, mybir.AxisListType.X,
                                            ALU.add)
                    mean = fc.tile([128, 4], F32, tag="mean", bufs=2)
                    nc.vector.tensor_scalar_mul(mean[:], mu[:], 1.0 / 128.0)
                    ctr = fc.tile([128, 4 * 128], F32, tag="ctr", bufs=2)
                    ctr_v = ctr[:].rearrange("p (c d) -> p c d", d=128)
                    nc.vector.tensor_tensor(
                        ctr_v, fo_v, mean[:].unsqueeze(-1).broadcast_to(c3),
                        ALU.subtract)
                    sq = fc.tile([128, 4 * 128], F32, tag="sq", bufs=2)
                    sq_v = sq[:].rearrange("p (c d) -> p c d", d=128)
                    nc.vector.tensor_tensor(sq_v, ctr_v, ctr_v, ALU.mult)
                    vs = fc.tile([128, 4], F32, tag="vs", bufs=2)
                    nc.vector.tensor_reduce(vs[:], sq_v, mybir.AxisListType.X,
                                            ALU.add)
                    eps_t = fc.tile([128, 1], F32, tag="eps", bufs=2)
                    nc.vector.memset(eps_t[:], EPS)
                    std = fc.tile([128, 4], F32, tag="std", bufs=2)
                    nc.scalar.activation(std[:], vs[:], AF.Sqrt,
                                         scale=1.0 / 128.0, bias=eps_t[:])
                    rs = fc.tile([128, 4], F32, tag="rs", bufs=2)
                    nc.vector.reciprocal(rs[:], std[:])
                    nrm = fc.tile([128, 4 * 128], F32, tag="nrm", bufs=2)
                    nrm_v = nrm[:].rearrange("p (c d) -> p c d", d=128)
                    nc.vector.tensor_tensor(
                        nrm_v, ctr_v, rs[:].unsqueeze(-1).broadcast_to(c3),
                        ALU.mult)
                    if affine:
                        g1 = fc.tile([128, 4 * 128], F32, tag="g1", bufs=2)
                        g1_v = g1[:].rearrange("p (c d) -> p c d", d=128)
                        nc.vector.tensor_tensor(
                            g1_v, nrm_v,
                            gamma_bc[:].unsqueeze(1).broadcast_to(c3), ALU.mult)
                        g2 = fc.tile([128, 4 * 128], F32, tag="g2", bufs=2)
                        g2_v = g2[:].rearrange("p (c d) -> p c d", d=128)
                        nc.vector.tensor_tensor(
                            g2_v, g1_v,
                            beta_bc[:].unsqueeze(1).broadcast_to(c3), ALU.add)
                    else:
                        g2, g2_v = nrm, nrm_v
                    g2s.append((cs, g2, g2_v))
                    if hf == 0:
                        # preload the sigmoid table behind half 1's DVE math
                        nc.scalar.activation(dum[:], ebias[:], AF.Sigmoid,
                                             scale=0.0)
                # sigmoids last: table already resident
                for cs, g2, g2_v in g2s:
                    sig = fc.tile([128, 4 * 128], F32, tag="sig", bufs=2)
                    nc.scalar.activation(sig[:], g2[:], AF.Sigmoid)
                    sil = fc.tile([128, 4 * 128], F32, tag="sil", bufs=2)
                    sil_v = sil[:].rearrange("p (c d) -> p c d", d=128)
                    nc.vector.tensor_tensor(sil_v, g2_v, sig[:].rearrange(
                        "p (c d) -> p c d", d=128), ALU.mult)
                    fin = fc.tile([128, 4 * 128], F32, tag="fin", bufs=2)
                    fin_v = fin[:].rearrange("p (c d) -> p c d", d=128)
                    nc.vector.tensor_tensor(
                        fin_v, sil_v, nm[:, cs].unsqueeze(-1).broadcast_to(c3),
                        ALU.mult)
                    nc.sync.dma_start(d_out[:, cs, :], fin_v)

    nc.compile()
    return nc


@lru_cache(maxsize=2)
def _program(affine: bool = False):
    return _build_program(affine)


class _Executor:
    """Caches the jitted shard_map executable across kernel() calls."""

    def __init__(self, nc):
        import jax
        import concourse.mybir as mb
        from concourse import bass2jax
        from jax.sharding import Mesh, PartitionSpec
        from jax.experimental.shard_map import shard_map

        bass2jax.install_neuronx_cc_hook()
        self.jax = jax
        partition_name = (
            nc.partition_id_tensor.name if nc.partition_id_tensor else None
        )
        in_names, out_names, out_avals, zero_shapes = [], [], [], []
        for alloc in nc.m.functions[0].allocations:
            if not isinstance(alloc, mb.MemoryLocationSet):
                continue
            name = alloc.memorylocations[0].name
            if alloc.kind == "ExternalInput":
                if name != partition_name:
                    in_names.append(name)
            elif alloc.kind == "ExternalOutput":
                out_names.append(name)
                shape = tuple(alloc.tensor_shape)
                dtype = mb.dt.np(alloc.dtype)
                out_avals.append(jax.core.ShapedArray(shape, dtype))
                zero_shapes.append((shape, dtype))
        self.n_params = len(in_names)
        self.in_names = list(in_names)
        self.out_names = out_names
        self.out_avals = out_avals
        self.zero_shapes = zero_shapes
        all_in = in_names + out_names + ([partition_name] if partition_name else [])
        donate = tuple(range(self.n_params, self.n_params + len(out_names)))

        def _body(*args):
            operands = list(args)
            if partition_name is not None:
                operands.append(bass2jax.partition_id_tensor())
            return tuple(bass2jax._bass_exec_p.bind(
                *operands,
                out_avals=tuple(out_avals),
                in_names=tuple(all_in),
                out_names=tuple(out_names),
                lowering_input_output_aliases=(),
                sim_require_finite=True,
                sim_require_nnan=True,
                nc=nc,
            ))

        devices = jax.devices()[:NCORES]
        mesh = Mesh(np.asarray(devices), ("core",))
        n_in = self.n_params + len(out_names)
        self.sharded = jax.jit(
            shard_map(_body, mesh=mesh,
                      in_specs=(PartitionSpec("core"),) * n_in,
                      out_specs=(PartitionSpec("core"),) * len(out_names),
                      check_rep=False),
            donate_argnums=donate, keep_unused=True,
        )

    def concat_inputs(self, in_maps):
        return [
            np.concatenate([np.asarray(m[name]) for m in in_maps], axis=0)
            for name in self.in_names
        ]

    def zeros(self):
        return [np.zeros((NCORES * s[0], *s[1:]), d) for s, d in self.zero_shapes]

    def run(self, concat_in):
        out_arrs = self.sharded(*concat_in, *self.zeros())
        return out_arrs

    def split(self, out_arrs):
        return [
            {name: np.asarray(out_arrs[i]).reshape(NCORES, *self.out_avals[i].shape)[c]
             for i, name in enumerate(self.out_names)}
            for c in range(NCORES)
        ]


@lru_cache(maxsize=2)
def _executor(affine: bool = False):
    return _Executor(_program(affine))


def _prep_core_inputs(core, x, attn_mask, node_mask, wa_h, wvo_h, bo,
                      gamma, beta):
    b, half = core // 2, core % 2
    rsl = slice(half * NQ, (half + 1) * NQ)
    xb = np.ascontiguousarray(x[b])
    m = {}
    xbT = np.ascontiguousarray(xb.T)
    m["x8T"] = xbT.astype(ml_dtypes.float8_e4m3)
    m["x8"] = np.ascontiguousarray(
        xb.reshape(NMC, 128, 128).transpose(1, 0, 2)
    ).astype(ml_dtypes.float8_e4m3)
    m["xqT"] = np.ascontiguousarray(xb[rsl].T).astype(ml_dtypes.bfloat16)
    m["xres"] = np.ascontiguousarray(
        (xb[rsl] + bo).reshape(8, 128, 128).transpose(1, 0, 2)
    )
    mT = attn_mask[b].T[:, rsl]  # [2048 m, 1024 n] bool
    mTr = np.ascontiguousarray(mT.reshape(NMC, 128, NQ).transpose(1, 0, 2))
    m["maskA"] = np.where(mTr, 0xFF, 0).astype(np.uint8)
    m["maskB"] = np.where(mTr, C2, MASKNEG).astype(ml_dtypes.bfloat16)
    m["wa"] = wa_h
    m["wvo"] = wvo_h
    m["gb"] = np.ascontiguousarray(np.stack([gamma, beta]))
    m["nm"] = np.ascontiguousarray(
        node_mask[b, rsl].astype(np.float32).reshape(8, 128).T
    )
    return m


def kernel(x, attn_mask, node_mask, Wq, Wk, Wv, Wo, bo, gamma, beta):
    x = np.asarray(x, np.float32)
    attn_mask = np.asarray(attn_mask, bool)
    node_mask = np.asarray(node_mask, bool)
    Wq = np.asarray(Wq, np.float32)
    Wk = np.asarray(Wk, np.float32)
    Wv = np.asarray(Wv, np.float32)
    Wo = np.asarray(Wo, np.float32)
    bo = np.asarray(bo, np.float32)
    gamma = np.asarray(gamma, np.float32)
    beta = np.asarray(beta, np.float32)

    # host weight folding: A_h = Wq_h Wk_h^T (shipped transposed), Wvo_h = Wv_h Wo_h
    wa_h = np.empty((128, 8, 128), np.float32)
    wvo_h = np.empty((128, 8, 128), np.float32)
    for h in range(H):
        hsl = slice(h * D, (h + 1) * D)
        A = Wq[:, hsl] @ Wk[:, hsl].T          # [F, F]
        wa_h[:, h, :] = A                      # wa[f', h, f] = A[f', f]
        wvo_h[:, h, :] = Wv[:, hsl] @ Wo[hsl]  # [F, 128]
    wa_h = np.ascontiguousarray(wa_h).astype(ml_dtypes.bfloat16)
    wvo_h = np.ascontiguousarray(wvo_h).astype(ml_dtypes.bfloat16)

    affine = not (np.all(gamma == 1.0) and np.all(beta == 0.0))
    ex = _executor(affine)
    in_maps = [
        _prep_core_inputs(c, x, attn_mask, node_mask, wa_h, wvo_h, bo,
                          gamma, beta)
        for c in range(NCORES)
    ]
    results = ex.split(ex.run(ex.concat_inputs(in_maps)))
    out = np.empty((B, N, D), np.float32)
    for core in range(NCORES):
        b, half = core // 2, core % 2
        o = results[core]["out"]  # [128, 8, 128]
        out[b, half * NQ:(half + 1) * NQ] = (
            o.transpose(1, 0, 2).reshape(NQ, 128)
        )
    return out


# revision 29
# speedup vs baseline: 1.0680x; 1.0189x over previous
"""Dense GAT layer (attention + out-proj + residual + LayerNorm + SiLU + node mask)
as a fused Bass/Tile kernel on 8 Trainium2 NeuronCores.

Sharding: core = (b, half) with b = core//2, half = core%2. Each core computes
output rows [half*1024, (half+1)*1024) of batch b; the host concatenates row
blocks (no collectives).

Weight folding (host, fp32): A_h = Wq_h @ Wk_h^T and Wvo_h = Wv_h @ Wo_h.
Then scores S_h = x A_h x^T (no Q/K projections on device) and
y = sum_h (x^T P_h)^T Wvo_h (no V projection; attention applied to raw x).

Per-core pipeline (per head):
  kt[f',m] = A_h^T.T @ xT on PE (bf16, the only projection) -> ACT copies to
  fp8 SBUF. S^T[m,n] = kt.T @ xq8 as fp8 DoubleRow matmuls with stride-0
  broadcast of the dummy k-tile (computes 2*S at 2 cols/cycle; the 2 is
  folded into the exp scale).
  exp+mask into fp8 pt, split across engines per 512-query half:
    ACT halves: ACT Exp (fp8 out, bias -2) + DVE uint32 bitwise-AND against
    a packed {0x00,0xFF} mask.
    DVE halves: one fused scalar_tensor_tensor Schraudolph exp-to-fp8-bits
    (psum*C1 + maskbias -> saturating uint8), maskbias = C2 on valid pairs,
    -200 on masked pairs (saturates to fp8 0.0).
  U[f,n] = x8-chunk-pairs.T @ pt-pairs as fp8 DoubleRow (contraction 256 per
  pass, 2 cols/cycle); row-sums r[n] via 1-column DoubleRow matmuls.
  Normalization fused into U's PSUM evacuation: 1/r (DVE) -> PE transpose ->
  DMA deswizzle to a [1,1024] row -> PE rank-1 broadcast -> one DVE
  multiply-copy U*(1/r) -> bf16 SBUF (Us).
  Tail: out-proj accumulating all heads (stationary Us slices, moving Wvo),
  then residual + LayerNorm (rsqrt via Ln/Exp, same ACT table as the
  attention exp) + SiLU (single Sigmoid table switch) + node mask.
Softmax skips the row-max subtraction: scores ~N(0,1); with bias -2 the fp8
range (448) holds exp(s-2) for s < 8.1 (max |s| ~ 7 over this problem).
"""

import math
from functools import lru_cache

import ml_dtypes
import numpy as np

import concourse.bacc as bacc
import concourse.mybir as mybir
import concourse.tile as tile
from concourse import masks

B, N, F = 4, 2048, 128
H, D = 8, 128
NQ = 1024  # query rows per core
NCORES = 8
EPS = 1e-5
SCALE = 1.0 / math.sqrt(D)
EXPB = 2.0  # subtracted inside exp; cancels in softmax normalization
# Schraudolph-to-fp8e4m3 constants (arg = psum*C1 + C2, psum = 2*s_raw)
C1 = (8.0 / math.log(2.0)) * (SCALE / 2.0)
C2 = 56.0 - (8.0 / math.log(2.0)) * EXPB - 0.45
MASKNEG = -200.0

# exp engine per (head, query-half): 9 of 16 halves on ACT, 7 on DVE
ACT_HALVES = {(0, 0), (0, 1), (1, 0), (1, 1), (2, 0), (2, 1), (3, 0), (3, 1),
              (4, 0)}

F32 = mybir.dt.float32
BF16 = mybir.dt.bfloat16
FP8 = mybir.dt.float8e4
U8 = mybir.dt.uint8
U32 = mybir.dt.uint32
AF = mybir.ActivationFunctionType
ALU = mybir.AluOpType
PM = mybir.MatmulPerfMode

NMC = N // 128  # 16 m-chunks


def _build_program(affine: bool = False):
    nc = bacc.Bacc(
        "TRN2", target_bir_lowering=False, debug=False, num_devices=NCORES
    )
    d_x8T = nc.declare_dram_parameter("x8T", [F, N], FP8, isOutput=False)
    d_x8 = nc.declare_dram_parameter("x8", [128, NMC, 128], FP8, isOutput=False)
    d_xqT = nc.declare_dram_parameter("xqT", [F, NQ], BF16, isOutput=False)
    d_xres = nc.declare_dram_parameter("xres", [128, 8, 128], F32, isOutput=False)
    d_maskA = nc.declare_dram_parameter("maskA", [128, NMC, NQ], U8, isOutput=False)
    d_maskB = nc.declare_dram_parameter("maskB", [128, NMC, NQ], BF16, isOutput=False)
    d_wa = nc.declare_dram_parameter("wa", [128, 8, 128], BF16, isOutput=False)
    d_wvo = nc.declare_dram_parameter("wvo", [128, 8, 128], BF16, isOutput=False)
    d_gb = nc.declare_dram_parameter("gb", [2, 128], F32, isOutput=False)
    d_nm = nc.declare_dram_parameter("nm", [128, 8], F32, isOutput=False)
    d_out = nc.declare_dram_parameter("out", [128, 8, 128], F32, isOutput=True)

    with tile.TileContext(nc) as tc:
        with (
            tc.tile_pool(name="const", bufs=1) as const,
            tc.tile_pool(name="small", bufs=2) as sp,
        ):
            # DMA order: head 0's A-projection needs wa/xT first; attention
            # needs xq8/x8/maskA quickly; maskB only by the first DVE half.
            wa = const.tile([128, 8 * 128], BF16)
            wa_v = wa[:].rearrange("p (h f) -> p h f", f=128)
            nc.sync.dma_start(wa_v, d_wa[:])
            xqT = const.tile([128, NQ], BF16)
            nc.sync.dma_start(xqT[:], d_xqT[:])
            x8T = const.tile([128, N], FP8)
            maskA = const.tile([128, NMC * NQ], U8)
            maskA_v = maskA[:].rearrange("p (c n) -> p c n", n=NQ)
            maskA32 = maskA[:].bitcast(U32).rearrange("p (c n) -> p c n", n=NQ // 4)
            maskB = const.tile([128, NMC * NQ], BF16)
            maskB_v = maskB[:].rearrange("p (c n) -> p c n", n=NQ)
            # x with keys on partitions: x8[p, c, f] = x[c*128+p, f]
            x8 = const.tile([128, NMC * 128], FP8)
            x8_v = x8[:].rearrange("p (c f) -> p c f", f=128)
            # stream in first-need order: S stationary slices, AND-mask for
            # units 0-1, STT-mask for unit 1, AV stationary, then the rest.
            for j4 in range(4):
                nc.sync.dma_start(x8T[:, j4 * 512:(j4 + 1) * 512],
                                  d_x8T[:, j4 * 512:(j4 + 1) * 512])
            nc.sync.dma_start(maskA_v[:, 0:4, :], d_maskA[:, 0:4, :])
            nc.sync.dma_start(maskB_v[:, 2:4, :], d_maskB[:, 2:4, :])
            for j4 in range(4):
                nc.sync.dma_start(
                    x8_v[:, j4 * 4:(j4 + 1) * 4, :],
                    d_x8[:, j4 * 4:(j4 + 1) * 4, :])
            nc.sync.dma_start(maskA_v[:, 4:8, :], d_maskA[:, 4:8, :])
            nc.sync.dma_start(maskB_v[:, 4:8, :], d_maskB[:, 4:8, :])
            nc.sync.dma_start(maskA_v[:, 8:16, :], d_maskA[:, 8:16, :])
            nc.sync.dma_start(maskB_v[:, 8:16, :], d_maskB[:, 8:16, :])
            nc.sync.dma_start(maskB_v[:, 0:2, :], d_maskB[:, 0:2, :])

            wvo = const.tile([128, 8 * 128], BF16)
            wvo_v = wvo[:].rearrange("p (h d) -> p h d", d=128)
            nc.sync.dma_start(wvo_v, d_wvo[:])
            xres = const.tile([128, 8 * 128], F32)
            xres_v = xres[:].rearrange("p (c d) -> p c d", d=128)
            nc.sync.dma_start(xres_v, d_xres[:])
            if affine:
                gbg = const.tile([1, 128], F32)
                nc.sync.dma_start(gbg[:], d_gb[0:1, :])
                gbb = const.tile([1, 128], F32)
                nc.sync.dma_start(gbb[:], d_gb[1:2, :])
            nm = const.tile([128, 8], F32)
            nc.sync.dma_start(nm[:], d_nm[:])

            ident = const.tile([128, 128], BF16)
            masks.make_identity(nc, ident[:])
            ones1 = const.tile([1, 128], BF16)
            nc.vector.memset(ones1[:], 1.0)
            ones8 = const.tile([128, 1], FP8)
            nc.vector.memset(ones8[:], 1.0)
            ebias = const.tile([128, 1], F32)
            nc.vector.memset(ebias[:], -EXPB)

            # normalized U = x^T P / r for all heads: [f, (h, n)]
            Us = const.tile([128, H * NQ], BF16)
            Us_v = Us[:].rearrange("p (h n) -> p h n", n=NQ)

            if affine:
                gamma_bc = const.tile([128, 128], F32)
                beta_bc = const.tile([128, 128], F32)

            with (
                tc.tile_pool(name="hp", bufs=2) as hp,
                tc.tile_pool(name="ptp", bufs=2) as ptp,
                tc.tile_pool(name="prp", bufs=2) as prp,
                tc.tile_pool(name="ps_work", bufs=3, space="PSUM") as ps_work,
                tc.tile_pool(name="ps_u", bufs=1, space="PSUM") as ps_u,
                tc.tile_pool(name="ps_r", bufs=1, space="PSUM") as ps_r,
            ):
                def emit_qa(h):
                    # query projection qa^T[f,n] = A_h^T x_q^T (bf16)
                    qa8 = hp.tile([128, NQ], FP8, tag="qa8", name="qa8")
                    wtq = ps_work.tile([128, 1024], F32, tag="work", name="wtq")
                    for j in range(2):
                        nc.tensor.matmul(
                            wtq[:, j * 512:(j + 1) * 512], wa_v[:, h, :],
                            xqT[:, j * 512:(j + 1) * 512],
                            start=True, stop=True,
                        )
                    nc.scalar.copy(qa8[:], wtq[:])
                    return qa8

                qa_next = emit_qa(0)
                pend = {"avs1": None, "avs2": None, "chain": None, "mult": None}

                def flush(key):
                    if pend[key] is not None:
                        pend[key]()
                        pend[key] = None

                for h in range(H):
                    qa8 = qa_next

                    # --- attention per 512-query half ---
                    for qh in range(2):
                        qsl = slice(qh * 512, (qh + 1) * 512)
                        # exp-engine route per praw-unit (2 S-groups each):
                        # True = ACT exp + DVE AND; False = DVE Schraudolph
                        routes = ((True, False, True, True)
                                  if (2 * h + qh) % 2 == 0 else
                                  (True, False, True, False))
                        ptt = ptp.tile([128, NMC * 512], U8, tag="ptt")
                        ptt_u = ptt[:].rearrange("p (c n) -> p c n", n=512)
                        ptt_v = ptt[:].bitcast(FP8).rearrange(
                            "p (c n) -> p c n", n=512)
                        ptt32 = ptt[:].bitcast(U32).rearrange(
                            "p (c n) -> p c n", n=128)
                        navd = [0]
                        holder = {}

                        def get_uts(holder=holder):
                            # lazy: allocate at first-AV flush time so the
                            # bufs=1 pool rotation matches emission order
                            if "ut" not in holder:
                                holder["ut"] = ps_u.tile([128, 512], F32,
                                                         tag="ut", name="ut")
                                holder["rps"] = ps_r.tile(
                                    [128, 4], F32, tag="rps", name="rps",
                                    padded_shape=[128, 512])
                            return holder["ut"], holder["rps"]

                        def emit_s_group(g, qsl=qsl, qa8=qa8):
                            # m-chunks 2g, 2g+1; 2*S via stride-0 DoubleRow
                            sgt = ps_work.tile([128, 1024], F32, tag="work",
                                               name="sgt")
                            for c in range(2):
                                mc = 2 * g + c
                                lhsT = x8T[:, mc * 128:(mc + 1) * 128] \
                                    .unsqueeze(1).broadcast_to([128, 2, 128])
                                rhs = qa8[:, qsl].unsqueeze(1) \
                                    .broadcast_to([128, 2, 512])
                                nc.tensor.matmul(
                                    sgt[:, c * 512:(c + 1) * 512], lhsT, rhs,
                                    start=True, stop=True, perf_mode=PM.DoubleRow,
                                )
                            return sgt

                        def emit_av(g, get_uts=get_uts, ptt_v=ptt_v,
                                    navd=navd):
                            ut, rps = get_uts()
                            first = navd[0] == 0
                            last = navd[0] == 7
                            navd[0] += 1
                            nc.tensor.matmul(
                                ut[:],
                                x8_v[:, 2 * g:2 * g + 2, :],
                                ptt_v[:, 2 * g:2 * g + 2, :],
                                start=first, stop=last,
                                perf_mode=PM.DoubleRow,
                            )
                            onev = ones8[:].unsqueeze(1).broadcast_to([128, 2, 1])
                            for ns in range(4):
                                nc.tensor.matmul(
                                    rps[:, ns:ns + 1],
                                    ptt_v[:, 2 * g:2 * g + 2,
                                          ns * 128:(ns + 1) * 128],
                                    onev,
                                    start=(first and ns == 0),
                                    stop=(last and ns == 3),
                                    perf_mode=PM.DoubleRow,
                                )

                        sgt = emit_s_group(0)
                        praw = None
                        for g in range(8):
                            u = g // 2
                            if routes[u]:
                                if g % 2 == 0:
                                    praw = prp.tile([128, 2048], FP8, tag="praw")
                                nc.scalar.activation(
                                    praw[:, (g % 2) * 1024:(g % 2 + 1) * 1024],
                                    sgt[:], AF.Exp, scale=SCALE / 2.0,
                                    bias=ebias[:])
                            else:
                                nc.vector.scalar_tensor_tensor(
                                    ptt_u[:, 2 * g:2 * g + 2, :],
                                    sgt[:].rearrange("p (c n) -> p c n", n=512),
                                    C1,
                                    maskB_v[:, 2 * g:2 * g + 2, qsl],
                                    ALU.mult, ALU.add)
                            if g < 7:
                                sgt = emit_s_group(g + 1)
                            if routes[u] and g % 2 == 1:
                                nc.vector.tensor_tensor(
                                    ptt32[:, 4 * u:4 * u + 4, :],
                                    praw[:].bitcast(U32).rearrange(
                                        "p (c n) -> p c n", n=128),
                                    maskA32[:, 4 * u:4 * u + 4,
                                            qh * 128:(qh + 1) * 128],
                                    ALU.bitwise_and)
                            # staggered flush of the previous half's deferred
                            # work, so PE keeps feeding S-groups to ACT/DVE
                            # while the prior half's A@V and normalization
                            # execute in the gaps
                            if g == 0:
                                flush("avs1")
                            elif g == 1:
                                flush("avs2")
                            elif g == 4:
                                flush("chain")
                        # previous half's U*(1/r): as late as allowed (its
                        # successor AV batch flushes early next half)
                        flush("mult")

                        def mk_avs(lo, hi, emit_av=emit_av):
                            def f():
                                for g2 in range(lo, hi):
                                    emit_av(g2)
                            return f
                        pend["avs1"] = mk_avs(0, 4)
                        pend["avs2"] = mk_avs(4, 8)

                        def mk_chain(h=h, qh=qh, qsl=qsl, get_uts=get_uts):
                            def f():
                                ut, rps = get_uts()
                                rb = sp.tile([128, 4], BF16, tag="rb")
                                with nc.allow_low_precision(
                                        reason="1/rowsum to bf16"):
                                    nc.vector.reciprocal(rb[:], rps[:])
                                wtt = ps_work.tile([128, 1024], F32,
                                                   tag="work", name="wtt")
                                rT = wtt[0:4, 0:64].bitcast(BF16)  # [4, 128]
                                nc.tensor.matmul(rT, rb[:], ident[:],
                                                 is_transpose=True,
                                                 start=True, stop=True)
                                rTs = sp.tile([4, 128], BF16, tag="rTs")
                                nc.vector.tensor_copy(rTs[:], rT)
                                rrow = sp.tile([1, 512], BF16, tag="rrow")
                                nc.sync.dma_start(
                                    rrow[:].rearrange("p (a b) -> p a b", a=4),
                                    rTs[:])
                                rbc = sp.tile([128, 512], BF16, tag="rbc")
                                nc.gpsimd.partition_broadcast(rbc[:], rrow[:])

                                def fmult(h=h, qsl=qsl, ut=ut, rbc=rbc):
                                    nc.vector.tensor_tensor(
                                        Us_v[:, h, qsl], ut[:], rbc[:],
                                        ALU.mult)
                                pend["mult"] = fmult
                            return f
                        pend["chain"] = mk_chain()
                        if h == H - 1 and qh == 1:
                            flush("avs1")
                            flush("avs2")
                            flush("chain")
                            flush("mult")
                        # hoist next head's qa projection between halves so
                        # the head boundary has no serial PE->ACT chain
                        if qh == 0 and h + 1 < H:
                            qa_next = emit_qa(h + 1)

            # --- out-proj + residual + LayerNorm + SiLU + node mask ---
            with (
                tc.tile_pool(name="fc", bufs=1) as fc,
                tc.tile_pool(name="ps_o", bufs=1, space="PSUM") as ps_o,
            ):
                if affine:
                    gps = ps_o.tile([128, 256], F32, tag="gps")
                    nc.tensor.matmul(gps[:, 0:128], ones1[:], gbg[:],
                                     start=True, stop=True)
                    nc.tensor.matmul(gps[:, 128:256], ones1[:], gbb[:],
                                     start=True, stop=True)
                    nc.vector.tensor_copy(gamma_bc[:], gps[:, 0:128])
                    nc.vector.tensor_copy(beta_bc[:], gps[:, 128:256])

                po_all = ps_o.tile([128, 8 * 128], F32, tag="po")
                for c in range(8):
                    for h2 in range(H):
                        nc.tensor.matmul(
                            po_all[:, c * 128:(c + 1) * 128],
                            Us_v[:, h2, c * 128:(c + 1) * 128], wvo_v[:, h2, :],
                            start=(h2 == 0), stop=(h2 == H - 1),
                        )
                po_v = po_all[:].rearrange("p (c d) -> p c d", d=128)
                c3 = [128, 4, 128]
                # dummy 1-col activations: preload the Sqrt/Sigmoid ACT
                # tables while DVE chews the LayerNorm math
                dum = fc.tile([128, 1], F32, tag="dum")
                nc.scalar.activation(dum[:], ones1[0:1, 0:1].broadcast_to(
                    [128, 1]) if False else ebias[:], AF.Sqrt, scale=0.0)
                g2s = []
                for hf in range(2):
                    cs = slice(hf * 4, (hf + 1) * 4)
                    fo = fc.tile([128, 4 * 128], F32, tag="fo", bufs=2)
                    fo_v = fo[:].rearrange("p (c d) -> p c d", d=128)
                    nc.vector.tensor_tensor(fo_v, po_v[:, cs, :],
                                            xres_v[:, cs, :], ALU.add)
                    mu = fc.tile([128, 4], F32, tag="mu", bufs=2)
                    nc.vector.tensor_reduce(mu[:], fo_v, mybir.AxisListType.X,
                                            ALU.add)
                    mean = fc.tile([128, 4], F32, tag="mean", bufs=2)
                    nc.vector.tensor_scalar_mul(mean[:], mu[:], 1.0 / 128.0)
                    ctr = fc.tile([128, 4 * 128], F32, tag="ctr", bufs=2)
                    ctr_v = ctr[:].rearrange("p (c d) -> p c d", d=128)
                    nc.vector.tensor_tensor(
                        ctr_v, fo_v, mean[:].unsqueeze(-1).broadcast_to(c3),
                        ALU.subtract)
                    sq = fc.tile([128, 4 * 128], F32, tag="sq", bufs=2)
                    sq_v = sq[:].rearrange("p (c d) -> p c d", d=128)
                    nc.vector.tensor_tensor(sq_v, ctr_v, ctr_v, ALU.mult)
                    vs = fc.tile([128, 4], F32, tag="vs", bufs=2)
                    nc.vector.tensor_reduce(vs[:], sq_v, mybir.AxisListType.X,
                                            ALU.add)
                    eps_t = fc.tile([128, 1], F32, tag="eps", bufs=2)
                    nc.vector.memset(eps_t[:], EPS)
                    std = fc.tile([128, 4], F32, tag="std", bufs=2)
                    nc.scalar.activation(std[:], vs[:], AF.Sqrt,
                                         scale=1.0 / 128.0, bias=eps_t[:])
                    rs = fc.tile([128, 4], F32, tag="rs", bufs=2)
                    nc.vector.reciprocal(rs[:], std[:])
                    nrm = fc.tile([128, 4 * 128], F32, tag="nrm", bufs=2)
                    nrm_v = nrm[:].rearrange("p (c d) -> p c d", d=128)
                    nc.vector.tensor_tensor(
                        nrm_v, ctr_v, rs[:].unsqueeze(-1).broadcast_to(c3),
                        ALU.mult)
                    if affine:
                        g1 = fc.tile([128, 4 * 128], F32, tag="g1", bufs=2)
                        g1_v = g1[:].rearrange("p (c d) -> p c d", d=128)
                        nc.vector.tensor_tensor(
                            g1_v, nrm_v,
                            gamma_bc[:].unsqueeze(1).broadcast_to(c3), ALU.mult)
                        g2 = fc.tile([128, 4 * 128], F32, tag="g2", bufs=2)
                        g2_v = g2[:].rearrange("p (c d) -> p c d", d=128)
                        nc.vector.tensor_tensor(
                            g2_v, g1_v,
                            beta_bc[:].unsqueeze(1).broadcast_to(c3), ALU.add)
                    else:
                        g2, g2_v = nrm, nrm_v
                    g2s.append((cs, g2, g2_v))
                    if hf == 0:
                        # preload the sigmoid table behind half 1's DVE math
                        nc.scalar.activation(dum[:], ebias[:], AF.Sigmoid,
                                             scale=0.0)
                # sigmoids last: table already resident
                for cs, g2, g2_v in g2s:
                    sig = fc.tile([128, 4 * 128], F32, tag="sig", bufs=2)
                    nc.scalar.activation(sig[:], g2[:], AF.Sigmoid)
                    sil = fc.tile([128, 4 * 128], F32, tag="sil", bufs=2)
                    sil_v = sil[:].rearrange("p (c d) -> p c d", d=128)
                    nc.vector.tensor_tensor(sil_v, g2_v, sig[:].rearrange(
                        "p (c d) -> p c d", d=128), ALU.mult)
                    fin = fc.tile([128, 4 * 128], F32, tag="fin", bufs=2)
                    fin_v = fin[:].rearrange("p (c d) -> p c d", d=128)
                    nc.vector.tensor_tensor(
                        fin_v, sil_v, nm[:, cs].unsqueeze(-1).broadcast_to(c3),
                        ALU.mult)
                    nc.sync.dma_start(d_out[:, cs, :], fin_v)

    nc.compile()
    return nc


@lru_cache(maxsize=2)
def _program(affine: bool = False):
    return _build_program(affine)


class _Executor:
    """Caches the jitted shard_map executable across kernel() calls."""

    def __init__(self, nc):
        import jax
        import concourse.mybir as mb
        from concourse import bass2jax
        from jax.sharding import Mesh, PartitionSpec
        from jax.experimental.shard_map import shard_map

        bass2jax.install_neuronx_cc_hook()
        self.jax = jax
        partition_name = (
            nc.partition_id_tensor.name if nc.partition_id_tensor else None
        )
        in_names, out_names, out_avals, zero_shapes = [], [], [], []
        for alloc in nc.m.functions[0].allocations:
            if not isinstance(alloc, mb.MemoryLocationSet):
                continue
            name = alloc.memorylocations[0].name
            if alloc.kind == "ExternalInput":
                if name != partition_name:
                    in_names.append(name)
            elif alloc.kind == "ExternalOutput":
                out_names.append(name)
                shape = tuple(alloc.tensor_shape)
                dtype = mb.dt.np(alloc.dtype)
                out_avals.append(jax.core.ShapedArray(shape, dtype))
                zero_shapes.append((shape, dtype))
        self.n_params = len(in_names)
        self.in_names = list(in_names)
        self.out_names = out_names
        self.out_avals = out_avals
        self.zero_shapes = zero_shapes
        all_in = in_names + out_names + ([partition_name] if partition_name else [])
        donate = tuple(range(self.n_params, self.n_params + len(out_names)))

        def _body(*args):
            operands = list(args)
            if partition_name is not None:
                operands.append(bass2jax.partition_id_tensor())
            return tuple(bass2jax._bass_exec_p.bind(
                *operands,
                out_avals=tuple(out_avals),
                in_names=tuple(all_in),
                out_names=tuple(out_names),
                lowering_input_output_aliases=(),
                sim_require_finite=True,
                sim_require_nnan=True,
                nc=nc,
            ))

        devices = jax.devices()[:NCORES]
        mesh = Mesh(np.asarray(devices), ("core",))
        n_in = self.n_params + len(out_names)
        self.sharded = jax.jit(
            shard_map(_body, mesh=mesh,
                      in_specs=(PartitionSpec("core"),) * n_in,
                      out_specs=(PartitionSpec("core"),) * len(out_names),
                      check_rep=False),
            donate_argnums=donate, keep_unused=True,
        )

    def concat_inputs(self, in_maps):
        return [
            np.concatenate([np.asarray(m[name]) for m in in_maps], axis=0)
            for name in self.in_names
        ]

    def zeros(self):
        return [np.zeros((NCORES * s[0], *s[1:]), d) for s, d in self.zero_shapes]

    def run(self, concat_in):
        out_arrs = self.sharded(*concat_in, *self.zeros())
        return out_arrs

    def split(self, out_arrs):
        return [
            {name: np.asarray(out_arrs[i]).reshape(NCORES, *self.out_avals[i].shape)[c]
             for i, name in enumerate(self.out_names)}
            for c in range(NCORES)
        ]


@lru_cache(maxsize=2)
def _executor(affine: bool = False):
    return _Executor(_program(affine))


def _prep_core_inputs(core, x, attn_mask, node_mask, wa_h, wvo_h, bo,
                      gamma, beta):
    b, half = core // 2, core % 2
    rsl = slice(half * NQ, (half + 1) * NQ)
    xb = np.ascontiguousarray(x[b])
    m = {}
    xbT = np.ascontiguousarray(xb.T)
    m["x8T"] = xbT.astype(ml_dtypes.float8_e4m3)
    m["x8"] = np.ascontiguousarray(
        xb.reshape(NMC, 128, 128).transpose(1, 0, 2)
    ).astype(ml_dtypes.float8_e4m3)
    m["xqT"] = np.ascontiguousarray(xb[rsl].T).astype(ml_dtypes.bfloat16)
    m["xres"] = np.ascontiguousarray(
        (xb[rsl] + bo).reshape(8, 128, 128).transpose(1, 0, 2)
    )
    mT = attn_mask[b].T[:, rsl]  # [2048 m, 1024 n] bool
    mTr = np.ascontiguousarray(mT.reshape(NMC, 128, NQ).transpose(1, 0, 2))
    m["maskA"] = np.where(mTr, 0xFF, 0).astype(np.uint8)
    m["maskB"] = np.where(mTr, C2, MASKNEG).astype(ml_dtypes.bfloat16)
    m["wa"] = wa_h
    m["wvo"] = wvo_h
    m["gb"] = np.ascontiguousarray(np.stack([gamma, beta]))
    m["nm"] = np.ascontiguousarray(
        node_mask[b, rsl].astype(np.float32).reshape(8, 128).T
    )
    return m


def kernel(x, attn_mask, node_mask, Wq, Wk, Wv, Wo, bo, gamma, beta):
    x = np.asarray(x, np.float32)
    attn_mask = np.asarray(attn_mask, bool)
    node_mask = np.asarray(node_mask, bool)
    Wq = np.asarray(Wq, np.float32)
    Wk = np.asarray(Wk, np.float32)
    Wv = np.asarray(Wv, np.float32)
    Wo = np.asarray(Wo, np.float32)
    bo = np.asarray(bo, np.float32)
    gamma = np.asarray(gamma, np.float32)
    beta = np.asarray(beta, np.float32)

    # host weight folding: A_h = Wq_h Wk_h^T (shipped transposed), Wvo_h = Wv_h Wo_h
    wa_h = np.empty((128, 8, 128), np.float32)
    wvo_h = np.empty((128, 8, 128), np.float32)
    for h in range(H):
        hsl = slice(h * D, (h + 1) * D)
        A = Wq[:, hsl] @ Wk[:, hsl].T          # [F, F]
        wa_h[:, h, :] = A                      # wa[f', h, f] = A[f', f]
        wvo_h[:, h, :] = Wv[:, hsl] @ Wo[hsl]  # [F, 128]
    wa_h = np.ascontiguousarray(wa_h).astype(ml_dtypes.bfloat16)
    wvo_h = np.ascontiguousarray(wvo_h).astype(ml_dtypes.bfloat16)

    affine = not (np.all(gamma == 1.0) and np.all(beta == 0.0))
    ex = _executor(affine)
    in_maps = [
        _prep_core_inputs(c, x, attn_mask, node_mask, wa_h, wvo_h, bo,
                          gamma, beta)
        for c in range(NCORES)
    ]
    results = ex.split(ex.run(ex.concat_inputs(in_maps)))
    out = np.empty((B, N, D), np.float32)
    for core in range(NCORES):
        b, half = core // 2, core % 2
        o = results[core]["out"]  # [128, 8, 128]
        out[b, half * NQ:(half + 1) * NQ] = (
            o.transpose(1, 0, 2).reshape(NQ, 128)
        )
    return out


# revision 30
# speedup vs baseline: 1.1313x; 1.0593x over previous
"""Dense GAT layer (attention + out-proj + residual + LayerNorm + SiLU + node mask)
as a fused Bass/Tile kernel on 8 Trainium2 NeuronCores.

Sharding: core = (b, half) with b = core//2, half = core%2. Each core computes
output rows [half*1024, (half+1)*1024) of batch b; the host concatenates row
blocks (no collectives).

Weight folding (host, fp32): A_h = Wq_h @ Wk_h^T and Wvo_h = Wv_h @ Wo_h.
Then scores S_h = x A_h x^T (no Q/K projections on device) and
y = sum_h (x^T P_h)^T Wvo_h (no V projection; attention applied to raw x).

Per-core pipeline (per head):
  kt[f',m] = A_h^T.T @ xT on PE (bf16, the only projection) -> ACT copies to
  fp8 SBUF. S^T[m,n] = kt.T @ xq8 as fp8 DoubleRow matmuls with stride-0
  broadcast of the dummy k-tile (computes 2*S at 2 cols/cycle; the 2 is
  folded into the exp scale).
  exp+mask into fp8 pt, split across engines per 512-query half:
    ACT halves: ACT Exp (fp8 out, bias -2) + DVE uint32 bitwise-AND against
    a packed {0x00,0xFF} mask.
    DVE halves: one fused scalar_tensor_tensor Schraudolph exp-to-fp8-bits
    (psum*C1 + maskbias -> saturating uint8), maskbias = C2 on valid pairs,
    -200 on masked pairs (saturates to fp8 0.0).
  U[f,n] = x8-chunk-pairs.T @ pt-pairs as fp8 DoubleRow (contraction 256 per
  pass, 2 cols/cycle); row-sums r[n] via 1-column DoubleRow matmuls.
  Normalization fused into U's PSUM evacuation: 1/r (DVE) -> PE transpose ->
  DMA deswizzle to a [1,1024] row -> PE rank-1 broadcast -> one DVE
  multiply-copy U*(1/r) -> bf16 SBUF (Us).
  Tail: out-proj accumulating all heads (stationary Us slices, moving Wvo),
  then residual + LayerNorm (rsqrt via Ln/Exp, same ACT table as the
  attention exp) + SiLU (single Sigmoid table switch) + node mask.
Softmax skips the row-max subtraction: scores ~N(0,1); with bias -2 the fp8
range (448) holds exp(s-2) for s < 8.1 (max |s| ~ 7 over this problem).
"""

import math
from functools import lru_cache

import ml_dtypes
import numpy as np

import concourse.bacc as bacc
import concourse.mybir as mybir
import concourse.tile as tile
from concourse import masks

B, N, F = 4, 2048, 128
H, D = 8, 128
NQ = 1024  # query rows per core
NCORES = 8
EPS = 1e-5
SCALE = 1.0 / math.sqrt(D)
EXPB = 2.0  # subtracted inside exp; cancels in softmax normalization
# Schraudolph-to-fp8e4m3 constants (arg = psum*C1 + C2, psum = 2*s_raw)
C1 = (8.0 / math.log(2.0)) * (SCALE / 2.0)
C2 = 56.0 - (8.0 / math.log(2.0)) * EXPB - 0.45
MASKNEG = -200.0

# exp engine per (head, query-half): 9 of 16 halves on ACT, 7 on DVE
ACT_HALVES = {(0, 0), (0, 1), (1, 0), (1, 1), (2, 0), (2, 1), (3, 0), (3, 1),
              (4, 0)}

F32 = mybir.dt.float32
BF16 = mybir.dt.bfloat16
FP8 = mybir.dt.float8e4
U8 = mybir.dt.uint8
U32 = mybir.dt.uint32
AF = mybir.ActivationFunctionType
ALU = mybir.AluOpType
PM = mybir.MatmulPerfMode

NMC = N // 128  # 16 m-chunks


def _build_program(affine: bool = False):
    nc = bacc.Bacc(
        "TRN2", target_bir_lowering=False, debug=False, num_devices=NCORES
    )
    d_x8T = nc.declare_dram_parameter("x8T", [F, N], FP8, isOutput=False)
    d_x8 = nc.declare_dram_parameter("x8", [128, NMC, 128], FP8, isOutput=False)
    d_xqT = nc.declare_dram_parameter("xqT", [F, NQ], BF16, isOutput=False)
    d_xres = nc.declare_dram_parameter("xres", [128, 8, 128], F32, isOutput=False)
    d_maskA = nc.declare_dram_parameter("maskA", [128, NMC, NQ], U8, isOutput=False)
    d_maskB = nc.declare_dram_parameter("maskB", [128, NMC, NQ], BF16, isOutput=False)
    d_wa = nc.declare_dram_parameter("wa", [128, 8, 128], BF16, isOutput=False)
    d_wvo = nc.declare_dram_parameter("wvo", [128, 8, 128], BF16, isOutput=False)
    d_gb = nc.declare_dram_parameter("gb", [2, 128], F32, isOutput=False)
    d_nm = nc.declare_dram_parameter("nm", [128, 8], F32, isOutput=False)
    d_out = nc.declare_dram_parameter("out", [128, 8, 128], F32, isOutput=True)

    with tile.TileContext(nc) as tc:
        with (
            tc.tile_pool(name="const", bufs=1) as const,
            tc.tile_pool(name="small", bufs=2) as sp,
        ):
            # DMA order: head 0's A-projection needs wa/xT first; attention
            # needs xq8/x8/maskA quickly; maskB only by the first DVE half.
            wa = const.tile([128, 8 * 128], BF16)
            wa_v = wa[:].rearrange("p (h f) -> p h f", f=128)
            nc.sync.dma_start(wa_v, d_wa[:])
            xqT = const.tile([128, NQ], BF16)
            nc.sync.dma_start(xqT[:], d_xqT[:])
            x8T = const.tile([128, N], FP8)
            maskA = const.tile([128, NMC * NQ], U8)
            maskA_v = maskA[:].rearrange("p (c n) -> p c n", n=NQ)
            maskA32 = maskA[:].bitcast(U32).rearrange("p (c n) -> p c n", n=NQ // 4)
            maskB = const.tile([128, NMC * NQ], BF16)
            maskB_v = maskB[:].rearrange("p (c n) -> p c n", n=NQ)
            # x with keys on partitions: x8[p, c, f] = x[c*128+p, f]
            x8 = const.tile([128, NMC * 128], FP8)
            x8_v = x8[:].rearrange("p (c f) -> p c f", f=128)
            # stream in first-need order: S stationary slices, AND-mask for
            # units 0-1, STT-mask for unit 1, AV stationary, then the rest.
            for j4 in range(4):
                nc.sync.dma_start(x8T[:, j4 * 512:(j4 + 1) * 512],
                                  d_x8T[:, j4 * 512:(j4 + 1) * 512])
            nc.sync.dma_start(maskA_v[:, 0:4, :], d_maskA[:, 0:4, :])
            nc.sync.dma_start(maskB_v[:, 2:4, :], d_maskB[:, 2:4, :])
            for j4 in range(4):
                nc.sync.dma_start(
                    x8_v[:, j4 * 4:(j4 + 1) * 4, :],
                    d_x8[:, j4 * 4:(j4 + 1) * 4, :])
            nc.sync.dma_start(maskA_v[:, 4:8, :], d_maskA[:, 4:8, :])
            nc.sync.dma_start(maskB_v[:, 4:8, :], d_maskB[:, 4:8, :])
            nc.sync.dma_start(maskA_v[:, 8:16, :], d_maskA[:, 8:16, :])
            nc.sync.dma_start(maskB_v[:, 8:16, :], d_maskB[:, 8:16, :])
            nc.sync.dma_start(maskB_v[:, 0:2, :], d_maskB[:, 0:2, :])

            wvo = const.tile([128, 8 * 128], BF16)
            wvo_v = wvo[:].rearrange("p (h d) -> p h d", d=128)
            nc.sync.dma_start(wvo_v, d_wvo[:])
            xres = const.tile([128, 8 * 128], F32)
            xres_v = xres[:].rearrange("p (c d) -> p c d", d=128)
            nc.sync.dma_start(xres_v, d_xres[:])
            if affine:
                gbg = const.tile([1, 128], F32)
                nc.sync.dma_start(gbg[:], d_gb[0:1, :])
                gbb = const.tile([1, 128], F32)
                nc.sync.dma_start(gbb[:], d_gb[1:2, :])
            nm = const.tile([128, 8], F32)
            nc.sync.dma_start(nm[:], d_nm[:])

            ident = const.tile([128, 128], BF16)
            masks.make_identity(nc, ident[:])
            ones1 = const.tile([1, 128], BF16)
            nc.vector.memset(ones1[:], 1.0)
            ones8 = const.tile([128, 1], FP8)
            nc.vector.memset(ones8[:], 1.0)
            ebias = const.tile([128, 1], F32)
            nc.vector.memset(ebias[:], -EXPB)

            # normalized U = x^T P / r for all heads: [f, (h, n)]
            Us = const.tile([128, H * NQ], BF16)
            Us_v = Us[:].rearrange("p (h n) -> p h n", n=NQ)

            if affine:
                gamma_bc = const.tile([128, 128], F32)
                beta_bc = const.tile([128, 128], F32)

            with (
                tc.tile_pool(name="hp", bufs=2) as hp,
                tc.tile_pool(name="ptp", bufs=2) as ptp,
                tc.tile_pool(name="prp", bufs=2) as prp,
                tc.tile_pool(name="ps_work", bufs=3, space="PSUM") as ps_work,
                tc.tile_pool(name="ps_u", bufs=1, space="PSUM") as ps_u,
                tc.tile_pool(name="ps_r", bufs=1, space="PSUM") as ps_r,
            ):
                def emit_qa(h):
                    # query projection qa^T[f,n] = A_h^T x_q^T (bf16)
                    qa8 = hp.tile([128, NQ], FP8, tag="qa8", name="qa8")
                    wtq = ps_work.tile([128, 1024], F32, tag="work", name="wtq")
                    for j in range(2):
                        nc.tensor.matmul(
                            wtq[:, j * 512:(j + 1) * 512], wa_v[:, h, :],
                            xqT[:, j * 512:(j + 1) * 512],
                            start=True, stop=True,
                        )
                    nc.scalar.copy(qa8[:], wtq[:])
                    return qa8

                qa_next = emit_qa(0)
                pend = {"avs1": None, "avs2": None, "chain": None, "mult": None}

                def flush(key):
                    if pend[key] is not None:
                        pend[key]()
                        pend[key] = None

                for h in range(H):
                    qa8 = qa_next

                    # --- attention per 512-query half ---
                    for qh in range(2):
                        qsl = slice(qh * 512, (qh + 1) * 512)
                        # exp-engine route per praw-unit (2 S-groups each):
                        # True = ACT exp + DVE AND; False = DVE Schraudolph
                        routes = ((True, False, True, True)
                                  if (2 * h + qh) % 2 == 0 else
                                  (True, False, True, False))
                        if h == 0 and qh == 0:
                            # all-ACT first half: no maskB dependency at start
                            routes = (True, True, True, True)
                        ptt = ptp.tile([128, NMC * 512], U8, tag="ptt")
                        ptt_u = ptt[:].rearrange("p (c n) -> p c n", n=512)
                        ptt_v = ptt[:].bitcast(FP8).rearrange(
                            "p (c n) -> p c n", n=512)
                        ptt32 = ptt[:].bitcast(U32).rearrange(
                            "p (c n) -> p c n", n=128)
                        navd = [0]
                        holder = {}

                        def get_uts(holder=holder):
                            # lazy: allocate at first-AV flush time so the
                            # bufs=1 pool rotation matches emission order
                            if "ut" not in holder:
                                holder["ut"] = ps_u.tile([128, 512], F32,
                                                         tag="ut", name="ut")
                                holder["rps"] = ps_r.tile(
                                    [128, 4], F32, tag="rps", name="rps",
                                    padded_shape=[128, 512])
                            return holder["ut"], holder["rps"]

                        def emit_s_group(g, qsl=qsl, qa8=qa8):
                            # m-chunks 2g, 2g+1; 2*S via stride-0 DoubleRow
                            sgt = ps_work.tile([128, 1024], F32, tag="work",
                                               name="sgt")
                            for c in range(2):
                                mc = 2 * g + c
                                lhsT = x8T[:, mc * 128:(mc + 1) * 128] \
                                    .unsqueeze(1).broadcast_to([128, 2, 128])
                                rhs = qa8[:, qsl].unsqueeze(1) \
                                    .broadcast_to([128, 2, 512])
                                nc.tensor.matmul(
                                    sgt[:, c * 512:(c + 1) * 512], lhsT, rhs,
                                    start=True, stop=True, perf_mode=PM.DoubleRow,
                                )
                            return sgt

                        def emit_av(g, get_uts=get_uts, ptt_v=ptt_v,
                                    navd=navd):
                            ut, rps = get_uts()
                            first = navd[0] == 0
                            last = navd[0] == 7
                            navd[0] += 1
                            nc.tensor.matmul(
                                ut[:],
                                x8_v[:, 2 * g:2 * g + 2, :],
                                ptt_v[:, 2 * g:2 * g + 2, :],
                                start=first, stop=last,
                                perf_mode=PM.DoubleRow,
                            )
                            onev = ones8[:].unsqueeze(1).broadcast_to([128, 2, 1])
                            for ns in range(4):
                                nc.tensor.matmul(
                                    rps[:, ns:ns + 1],
                                    ptt_v[:, 2 * g:2 * g + 2,
                                          ns * 128:(ns + 1) * 128],
                                    onev,
                                    start=(first and ns == 0),
                                    stop=(last and ns == 3),
                                    perf_mode=PM.DoubleRow,
                                )

                        sgt = emit_s_group(0)
                        praw = None
                        for g in range(8):
                            u = g // 2
                            if routes[u]:
                                if g % 2 == 0:
                                    praw = prp.tile([128, 2048], FP8, tag="praw")
                                nc.scalar.activation(
                                    praw[:, (g % 2) * 1024:(g % 2 + 1) * 1024],
                                    sgt[:], AF.Exp, scale=SCALE / 2.0,
                                    bias=ebias[:])
                            else:
                                nc.vector.scalar_tensor_tensor(
                                    ptt_u[:, 2 * g:2 * g + 2, :],
                                    sgt[:].rearrange("p (c n) -> p c n", n=512),
                                    C1,
                                    maskB_v[:, 2 * g:2 * g + 2, qsl],
                                    ALU.mult, ALU.add)
                            if g < 7:
                                sgt = emit_s_group(g + 1)
                            if routes[u] and g % 2 == 1:
                                nc.vector.tensor_tensor(
                                    ptt32[:, 4 * u:4 * u + 4, :],
                                    praw[:].bitcast(U32).rearrange(
                                        "p (c n) -> p c n", n=128),
                                    maskA32[:, 4 * u:4 * u + 4,
                                            qh * 128:(qh + 1) * 128],
                                    ALU.bitwise_and)
                            # staggered flush of the previous half's deferred
                            # work, so PE keeps feeding S-groups to ACT/DVE
                            # while the prior half's A@V and normalization
                            # execute in the gaps
                            if g == 0:
                                flush("mult")
                                flush("avs1")
                            elif g == 1:
                                flush("avs2")
                            elif g == 6:
                                flush("chain")

                        def mk_avs(lo, hi, emit_av=emit_av):
                            def f():
                                for g2 in range(lo, hi):
                                    emit_av(g2)
                            return f
                        pend["avs1"] = mk_avs(0, 4)
                        pend["avs2"] = mk_avs(4, 8)

                        def mk_chain(h=h, qh=qh, qsl=qsl, get_uts=get_uts):
                            def f():
                                ut, rps = get_uts()
                                rb = sp.tile([128, 4], BF16, tag="rb")
                                with nc.allow_low_precision(
                                        reason="1/rowsum to bf16"):
                                    nc.vector.reciprocal(rb[:], rps[:])
                                wtt = ps_work.tile([128, 1024], F32,
                                                   tag="work", name="wtt")
                                rT = wtt[0:4, 0:64].bitcast(BF16)  # [4, 128]
                                nc.tensor.matmul(rT, rb[:], ident[:],
                                                 is_transpose=True,
                                                 start=True, stop=True)
                                rTs = sp.tile([4, 128], BF16, tag="rTs")
                                nc.vector.tensor_copy(rTs[:], rT)
                                rrow = sp.tile([1, 512], BF16, tag="rrow")
                                nc.sync.dma_start(
                                    rrow[:].rearrange("p (a b) -> p a b", a=4),
                                    rTs[:])
                                rbc = sp.tile([128, 512], BF16, tag="rbc")
                                nc.gpsimd.partition_broadcast(rbc[:], rrow[:])

                                def fmult(h=h, qsl=qsl, ut=ut, rbc=rbc):
                                    nc.vector.tensor_tensor(
                                        Us_v[:, h, qsl], ut[:], rbc[:],
                                        ALU.mult)
                                pend["mult"] = fmult
                            return f
                        pend["chain"] = mk_chain()
                        if h == H - 1 and qh == 1:
                            flush("avs1")
                            flush("avs2")
                            flush("chain")
                            flush("mult")
                        # hoist next head's qa projection between halves so
                        # the head boundary has no serial PE->ACT chain
                        if qh == 0 and h + 1 < H:
                            qa_next = emit_qa(h + 1)

            # --- out-proj + residual + LayerNorm + SiLU + node mask ---
            with (
                tc.tile_pool(name="fc", bufs=1) as fc,
                tc.tile_pool(name="ps_o", bufs=1, space="PSUM") as ps_o,
            ):
                if affine:
                    gps = ps_o.tile([128, 256], F32, tag="gps")
                    nc.tensor.matmul(gps[:, 0:128], ones1[:], gbg[:],
                                     start=True, stop=True)
                    nc.tensor.matmul(gps[:, 128:256], ones1[:], gbb[:],
                                     start=True, stop=True)
                    nc.vector.tensor_copy(gamma_bc[:], gps[:, 0:128])
                    nc.vector.tensor_copy(beta_bc[:], gps[:, 128:256])

                po_all = ps_o.tile([128, 8 * 128], F32, tag="po")
                for c in range(8):
                    for h2 in range(H):
                        nc.tensor.matmul(
                            po_all[:, c * 128:(c + 1) * 128],
                            Us_v[:, h2, c * 128:(c + 1) * 128], wvo_v[:, h2, :],
                            start=(h2 == 0), stop=(h2 == H - 1),
                        )
                po_v = po_all[:].rearrange("p (c d) -> p c d", d=128)
                c3 = [128, 4, 128]
                # dummy 1-col activations: preload the Sqrt/Sigmoid ACT
                # tables while DVE chews the LayerNorm math
                dum = fc.tile([128, 1], F32, tag="dum")
                nc.scalar.activation(dum[:], ones1[0:1, 0:1].broadcast_to(
                    [128, 1]) if False else ebias[:], AF.Sqrt, scale=0.0)
                g2s = []
                for hf in range(2):
                    cs = slice(hf * 4, (hf + 1) * 4)
                    fo = fc.tile([128, 4 * 128], F32, tag="fo", bufs=2)
                    fo_v = fo[:].rearrange("p (c d) -> p c d", d=128)
                    nc.vector.tensor_tensor(fo_v, po_v[:, cs, :],
                                            xres_v[:, cs, :], ALU.add)
                    mu = fc.tile([128, 4], F32, tag="mu", bufs=2)
                    nc.vector.tensor_reduce(mu[:], fo_v, mybir.AxisListType.X,
                                            ALU.add)
                    mean = fc.tile([128, 4], F32, tag="mean", bufs=2)
                    nc.vector.tensor_scalar_mul(mean[:], mu[:], 1.0 / 128.0)
                    ctr = fc.tile([128, 4 * 128], F32, tag="ctr", bufs=2)
                    ctr_v = ctr[:].rearrange("p (c d) -> p c d", d=128)
                    nc.vector.tensor_tensor(
                        ctr_v, fo_v, mean[:].unsqueeze(-1).broadcast_to(c3),
                        ALU.subtract)
                    sq = fc.tile([128, 4 * 128], F32, tag="sq", bufs=2)
                    sq_v = sq[:].rearrange("p (c d) -> p c d", d=128)
                    nc.vector.tensor_tensor(sq_v, ctr_v, ctr_v, ALU.mult)
                    vs = fc.tile([128, 4], F32, tag="vs", bufs=2)
                    nc.vector.tensor_reduce(vs[:], sq_v, mybir.AxisListType.X,
                                            ALU.add)
                    eps_t = fc.tile([128, 1], F32, tag="eps", bufs=2)
                    nc.vector.memset(eps_t[:], EPS)
                    std = fc.tile([128, 4], F32, tag="std", bufs=2)
                    nc.scalar.activation(std[:], vs[:], AF.Sqrt,
                                         scale=1.0 / 128.0, bias=eps_t[:])
                    rs = fc.tile([128, 4], F32, tag="rs", bufs=2)
                    nc.vector.reciprocal(rs[:], std[:])
                    nrm = fc.tile([128, 4 * 128], F32, tag="nrm", bufs=2)
                    nrm_v = nrm[:].rearrange("p (c d) -> p c d", d=128)
                    nc.vector.tensor_tensor(
                        nrm_v, ctr_v, rs[:].unsqueeze(-1).broadcast_to(c3),
                        ALU.mult)
                    if affine:
                        g1 = fc.tile([128, 4 * 128], F32, tag="g1", bufs=2)
                        g1_v = g1[:].rearrange("p (c d) -> p c d", d=128)
                        nc.vector.tensor_tensor(
                            g1_v, nrm_v,
                            gamma_bc[:].unsqueeze(1).broadcast_to(c3), ALU.mult)
                        g2 = fc.tile([128, 4 * 128], F32, tag="g2", bufs=2)
                        g2_v = g2[:].rearrange("p (c d) -> p c d", d=128)
                        nc.vector.tensor_tensor(
                            g2_v, g1_v,
                            beta_bc[:].unsqueeze(1).broadcast_to(c3), ALU.add)
                    else:
                        g2, g2_v = nrm, nrm_v
                    g2s.append((cs, g2, g2_v))
                    if hf == 0:
                        # preload the sigmoid table behind half 1's DVE math
                        nc.scalar.activation(dum[:], ebias[:], AF.Sigmoid,
                                             scale=0.0)
                # sigmoids last: table already resident
                for cs, g2, g2_v in g2s:
                    sig = fc.tile([128, 4 * 128], F32, tag="sig", bufs=2)
                    nc.scalar.activation(sig[:], g2[:], AF.Sigmoid)
                    sil = fc.tile([128, 4 * 128], F32, tag="sil", bufs=2)
                    sil_v = sil[:].rearrange("p (c d) -> p c d", d=128)
                    nc.vector.tensor_tensor(sil_v, g2_v, sig[:].rearrange(
                        "p (c d) -> p c d", d=128), ALU.mult)
                    fin = fc.tile([128, 4 * 128], F32, tag="fin", bufs=2)
                    fin_v = fin[:].rearrange("p (c d) -> p c d", d=128)
                    nc.vector.tensor_tensor(
                        fin_v, sil_v, nm[:, cs].unsqueeze(-1).broadcast_to(c3),
                        ALU.mult)
                    nc.sync.dma_start(d_out[:, cs, :], fin_v)

    nc.compile()
    return nc


@lru_cache(maxsize=2)
def _program(affine: bool = False):
    return _build_program(affine)


class _Executor:
    """Caches the jitted shard_map executable across kernel() calls."""

    def __init__(self, nc):
        import jax
        import concourse.mybir as mb
        from concourse import bass2jax
        from jax.sharding import Mesh, PartitionSpec
        from jax.experimental.shard_map import shard_map

        bass2jax.install_neuronx_cc_hook()
        self.jax = jax
        partition_name = (
            nc.partition_id_tensor.name if nc.partition_id_tensor else None
        )
        in_names, out_names, out_avals, zero_shapes = [], [], [], []
        for alloc in nc.m.functions[0].allocations:
            if not isinstance(alloc, mb.MemoryLocationSet):
                continue
            name = alloc.memorylocations[0].name
            if alloc.kind == "ExternalInput":
                if name != partition_name:
                    in_names.append(name)
            elif alloc.kind == "ExternalOutput":
                out_names.append(name)
                shape = tuple(alloc.tensor_shape)
                dtype = mb.dt.np(alloc.dtype)
                out_avals.append(jax.core.ShapedArray(shape, dtype))
                zero_shapes.append((shape, dtype))
        self.n_params = len(in_names)
        self.in_names = list(in_names)
        self.out_names = out_names
        self.out_avals = out_avals
        self.zero_shapes = zero_shapes
        all_in = in_names + out_names + ([partition_name] if partition_name else [])
        donate = tuple(range(self.n_params, self.n_params + len(out_names)))

        def _body(*args):
            operands = list(args)
            if partition_name is not None:
                operands.append(bass2jax.partition_id_tensor())
            return tuple(bass2jax._bass_exec_p.bind(
                *operands,
                out_avals=tuple(out_avals),
                in_names=tuple(all_in),
                out_names=tuple(out_names),
                lowering_input_output_aliases=(),
                sim_require_finite=True,
                sim_require_nnan=True,
                nc=nc,
            ))

        devices = jax.devices()[:NCORES]
        mesh = Mesh(np.asarray(devices), ("core",))
        n_in = self.n_params + len(out_names)
        self.sharded = jax.jit(
            shard_map(_body, mesh=mesh,
                      in_specs=(PartitionSpec("core"),) * n_in,
                      out_specs=(PartitionSpec("core"),) * len(out_names),
                      check_rep=False),
            donate_argnums=donate, keep_unused=True,
        )

    def concat_inputs(self, in_maps):
        return [
            np.concatenate([np.asarray(m[name]) for m in in_maps], axis=0)
            for name in self.in_names
        ]

    def zeros(self):
        return [np.zeros((NCORES * s[0], *s[1:]), d) for s, d in self.zero_shapes]

    def run(self, concat_in):
        out_arrs = self.sharded(*concat_in, *self.zeros())
        return out_arrs

    def split(self, out_arrs):
        return [
            {name: np.asarray(out_arrs[i]).reshape(NCORES, *self.out_avals[i].shape)[c]
             for i, name in enumerate(self.out_names)}
            for c in range(NCORES)
        ]


@lru_cache(maxsize=2)
def _executor(affine: bool = False):
    return _Executor(_program(affine))


def _prep_core_inputs(core, x, attn_mask, node_mask, wa_h, wvo_h, bo,
                      gamma, beta):
    b, half = core // 2, core % 2
    rsl = slice(half * NQ, (half + 1) * NQ)
    xb = np.ascontiguousarray(x[b])
    m = {}
    xbT = np.ascontiguousarray(xb.T)
    m["x8T"] = xbT.astype(ml_dtypes.float8_e4m3)
    m["x8"] = np.ascontiguousarray(
        xb.reshape(NMC, 128, 128).transpose(1, 0, 2)
    ).astype(ml_dtypes.float8_e4m3)
    m["xqT"] = np.ascontiguousarray(xb[rsl].T).astype(ml_dtypes.bfloat16)
    m["xres"] = np.ascontiguousarray(
        (xb[rsl] + bo).reshape(8, 128, 128).transpose(1, 0, 2)
    )
    mT = attn_mask[b].T[:, rsl]  # [2048 m, 1024 n] bool
    mTr = np.ascontiguousarray(mT.reshape(NMC, 128, NQ).transpose(1, 0, 2))
    m["maskA"] = np.where(mTr, 0xFF, 0).astype(np.uint8)
    m["maskB"] = np.where(mTr, C2, MASKNEG).astype(ml_dtypes.bfloat16)
    m["wa"] = wa_h
    m["wvo"] = wvo_h
    m["gb"] = np.ascontiguousarray(np.stack([gamma, beta]))
    m["nm"] = np.ascontiguousarray(
        node_mask[b, rsl].astype(np.float32).reshape(8, 128).T
    )
    return m


def kernel(x, attn_mask, node_mask, Wq, Wk, Wv, Wo, bo, gamma, beta):
    x = np.asarray(x, np.float32)
    attn_mask = np.asarray(attn_mask, bool)
    node_mask = np.asarray(node_mask, bool)
    Wq = np.asarray(Wq, np.float32)
    Wk = np.asarray(Wk, np.float32)
    Wv = np.asarray(Wv, np.float32)
    Wo = np.asarray(Wo, np.float32)
    bo = np.asarray(bo, np.float32)
    gamma = np.asarray(gamma, np.float32)
    beta = np.asarray(beta, np.float32)

    # host weight folding: A_h = Wq_h Wk_h^T (shipped transposed), Wvo_h = Wv_h Wo_h
    wa_h = np.empty((128, 8, 128), np.float32)
    wvo_h = np.empty((128, 8, 128), np.float32)
    for h in range(H):
        hsl = slice(h * D, (h + 1) * D)
        A = Wq[:, hsl] @ Wk[:, hsl].T          # [F, F]
        wa_h[:, h, :] = A                      # wa[f', h, f] = A[f', f]
        wvo_h[:, h, :] = Wv[:, hsl] @ Wo[hsl]  # [F, 128]
    wa_h = np.ascontiguousarray(wa_h).astype(ml_dtypes.bfloat16)
    wvo_h = np.ascontiguousarray(wvo_h).astype(ml_dtypes.bfloat16)

    affine = not (np.all(gamma == 1.0) and np.all(beta == 0.0))
    ex = _executor(affine)
    in_maps = [
        _prep_core_inputs(c, x, attn_mask, node_mask, wa_h, wvo_h, bo,
                          gamma, beta)
        for c in range(NCORES)
    ]
    results = ex.split(ex.run(ex.concat_inputs(in_maps)))
    out = np.empty((B, N, D), np.float32)
    for core in range(NCORES):
        b, half = core // 2, core % 2
        o = results[core]["out"]  # [128, 8, 128]
        out[b, half * NQ:(half + 1) * NQ] = (
            o.transpose(1, 0, 2).reshape(NQ, 128)
        )
    return out
